# revision 1
# baseline (speedup 1.0000x reference)
# CARAFE (content-aware reassembly) Trainium2 Bass kernel.
# Strategy: data-parallel over batch (8 items -> 8 NeuronCores). Per core:
#   - 1x1 compressor conv (PE matmul) + folded BN + SiLU (ACT)
#   - 3x3 encoder conv as 9 accumulating matmuls on padded t (PE) + folded BN
#   - exp + per-class (2x2 subpixel) normalization for the 25-way softmax
#   - 25-tap reassembly as per-pixel fused multiply-accumulate on DVE
#     (scalar_tensor_tensor), with partition-shifted x windows produced by
#     shift-matrix matmuls on PE and evicted by ACT
#   - output re-transposed to channel-major by PE and DMA'd out.
import sys
import numpy as np

for _p in ("/opt/trn_rl_repo",):
    if _p not in sys.path:
        sys.path.insert(0, _p)

B, C, Cm, E = 8, 192, 64, 100
H = W = 64
K, S = 5, 2
EPS = 1e-3
NT = 32            # pixel tiles (2 rows x 64 cols = 128 pixels each)
NB = 36            # xT_v row blocks of 128 (rows r in [-4, 68))

# shift-matrix offsets tau: S_tau[k, m] = 1 iff k == m + tau
TAUS = sorted({0, 1, 2, 62, 63, 64, 65, 66, 126, 127,
               -1, -2, -62, -63, -64, -65, -66, -126, -127})
TAU_IDX = {t: i for i, t in enumerate(TAUS)}

_prog_cache = {}


def _build_program(num_devices=8):
    import concourse.mybir as mybir
    import concourse.tile as tile
    from concourse import bacc
    from contextlib import ExitStack

    fp32 = mybir.dt.float32
    AL = mybir.AluOpType
    AF = mybir.ActivationFunctionType

    nc = bacc.Bacc("TRN2", target_bir_lowering=False, num_devices=num_devices)

    x_d = nc.dram_tensor("x", [C, H * W], fp32, kind="ExternalInput").ap()
    cw_d = nc.dram_tensor("cw", [C, Cm], fp32, kind="ExternalInput").ap()
    cb_d = nc.dram_tensor("cb", [Cm, 1], fp32, kind="ExternalInput").ap()
    ew_d = nc.dram_tensor("ew", [Cm, 9 * E], fp32, kind="ExternalInput").ap()
    eb_d = nc.dram_tensor("eb", [E, 1], fp32, kind="ExternalInput").ap()
    edge_d = nc.dram_tensor("edge", [128, K * K], fp32, kind="ExternalInput").ap()
    shm_d = nc.dram_tensor("shm", [128, len(TAUS) * 128], fp32, kind="ExternalInput").ap()
    out_d = nc.dram_tensor("out", [C, H, S, S * W], fp32, kind="ExternalOutput").ap()

    es = ExitStack()
    with tile.TileContext(nc) as tc:
        with es:
            _body(es, tc, nc, mybir, fp32, AL, AF,
                  x_d, cw_d, cb_d, ew_d, eb_d, edge_d, shm_d, out_d)
    nc.compile()
    return nc


def _body(es, tc, nc, mybir, fp32, AL, AF,
          x_d, cw_d, cb_d, ew_d, eb_d, edge_d, shm_d, out_d):
    consts = es.enter_context(tc.tile_pool(name="consts", bufs=1))
    big = es.enter_context(tc.tile_pool(name="big", bufs=1))

    cw0 = consts.tile([128, Cm], fp32, tag="cw0")
    cw1 = consts.tile([64, Cm], fp32, tag="cw1")
    cb = consts.tile([Cm, 1], fp32, tag="cb")
    ew = consts.tile([Cm, 9 * E], fp32, tag="ew")
    eb = consts.tile([E, 1], fp32, tag="eb")
    edge = consts.tile([128, K * K], fp32, tag="edge")
    shm = consts.tile([128, len(TAUS), 128], fp32, tag="shm")

    x0 = big.tile([128, H * W], fp32, tag="x0")
    x1 = big.tile([64, H * W], fp32, tag="x1")
    t_pad = big.tile([Cm, 66 * 66], fp32, tag="tpad")
    e_sb = big.tile([E, H * W], fp32, tag="esb")
    xT = big.tile([128, NB, C], fp32, tag="xT")
    maskT = big.tile([128, NT, E], fp32, tag="maskT")
    rsum = big.tile([128, NT, 4], fp32, tag="rsum")

    nc.sync.dma_start(out=cw0[:], in_=cw_d[0:128, :])
    nc.sync.dma_start(out=cw1[:], in_=cw_d[128:192, :])
    nc.sync.dma_start(out=cb[:], in_=cb_d)
    nc.sync.dma_start(out=ew[:], in_=ew_d)
    nc.sync.dma_start(out=eb[:], in_=eb_d)
    nc.sync.dma_start(out=edge[:], in_=edge_d)
    nc.sync.dma_start(out=shm[:].rearrange("p a b -> p (a b)"), in_=shm_d)
    for ck in range(8):
        c0 = ck * 512
        nc.sync.dma_start(out=x0[:, c0:c0 + 512], in_=x_d[0:128, c0:c0 + 512])
        nc.sync.dma_start(out=x1[:, c0:c0 + 512], in_=x_d[128:192, c0:c0 + 512])

    ident = shm[:, TAU_IDX[0], :]  # [128, 128] identity

    # zero borders of t_pad and the vertical zero blocks of xT
    nc.gpsimd.memset(t_pad[:], 0.0)
    nc.gpsimd.memset(xT[:, 0:2, :], 0.0)
    nc.gpsimd.memset(xT[:, NB - 2:NB, :], 0.0)

    # All PSUM pools stay open concurrently (8 banks total) so the stack
    # allocator never reuses addresses across phases (false deps would
    # serialize the phases).
    win_ps = es.enter_context(tc.tile_pool(name="win_ps", bufs=3, space="PSUM"))
    conv_ps = es.enter_context(tc.tile_pool(name="conv_ps", bufs=1, space="PSUM"))
    mt_ps = es.enter_context(tc.tile_pool(name="mt_ps", bufs=1, space="PSUM"))
    out_ps = es.enter_context(tc.tile_pool(name="out_ps", bufs=3, space="PSUM"))
    c1sg = es.enter_context(tc.tile_pool(name="c1sg", bufs=2))
    work = es.enter_context(tc.tile_pool(name="work", bufs=40))
    accp = es.enter_context(tc.tile_pool(name="accp", bufs=8))
    stagep = es.enter_context(tc.tile_pool(name="stagep", bufs=3))
    swp = es.enter_context(tc.tile_pool(name="swp", bufs=8))

    # ---- transpose x into xT (row blocks offset by +256 rows of zero pad) ----
    for pb in range(32):  # pixel blocks of 128
        p0 = pb * 128
        q = pb + 2
        pt0 = win_ps.tile([128, C], fp32, name="pt0", tag="winps")
        nc.tensor.transpose(pt0[:, 0:128], x0[:, p0:p0 + 128], ident)
        nc.scalar.copy(out=xT[:, q, 0:128], in_=pt0[:, 0:128])
        pt1 = win_ps.tile([128, C], fp32, name="pt1", tag="winps")
        nc.tensor.transpose(pt1[:, 0:64], x1[:, p0:p0 + 128], ident[0:64, 0:64])
        nc.scalar.copy(out=xT[:, q, 128:192], in_=pt1[:, 0:64])

    def conv1(nt):
        n0 = nt * 512
        ps = conv_ps.tile([E, 512], fp32, name="c1ps", tag="conv")
        nc.tensor.matmul(ps[0:Cm, :], cw0[:], x0[:, n0:n0 + 512], start=True, stop=False)
        nc.tensor.matmul(ps[0:Cm, :], cw1[:], x1[:, n0:n0 + 512], start=False, stop=True)
        # silu(y) = y*sigmoid(y) with y = ps + cb
        sg = c1sg.tile([Cm, 512], fp32, tag="sg")
        nc.scalar.activation(out=sg[:], in_=ps[0:Cm, :], func=AF.Sigmoid, bias=cb[:], scale=1.0)
        v = t_pad[:].rearrange("c (r z) -> c r z", z=66)[:, nt * 8 + 1: nt * 8 + 9, 1:65]
        nc.vector.scalar_tensor_tensor(
            v, ps[0:Cm, :].rearrange("c (r z) -> c r z", z=64), cb[:],
            sg[:].rearrange("c (r z) -> c r z", z=64), AL.add, AL.mult)

    def conv2(nt):
        r0 = nt * 8
        ps = conv_ps.tile([E, 512], fp32, name="c2ps", tag="conv")
        for tap in range(9):
            dy, dx = tap // 3, tap % 3
            rhs = t_pad[:].rearrange("c (r z) -> c r z", z=66)[:, r0 + dy: r0 + dy + 8, dx: dx + 64]
            nc.tensor.matmul(ps[:], ew[:, tap * E:(tap + 1) * E], rhs,
                             start=(tap == 0), stop=(tap == 8))
        nc.scalar.activation(out=e_sb[:, nt * 512:(nt + 1) * 512], in_=ps[:],
                             func=AF.Exp, bias=eb[:], scale=1.0)

    def mask_tile(ti):
        p0 = ti * 128
        pt = mt_ps.tile([128, E], fp32, name="mt", tag="mt")
        nc.tensor.transpose(pt[:], e_sb[:, p0:p0 + 128], ident[0:E, 0:E])
        nc.scalar.copy(out=maskT[:, ti, :], in_=pt[:])
        # maskT free layout: ch = ij*4 + cl
        v_cl_ij = maskT[:, ti, :].rearrange("p (ij cl) -> p cl ij", cl=4)
        s = rsum[:, ti, :]
        nc.vector.tensor_reduce(out=s, in_=v_cl_ij, axis=mybir.AxisListType.X, op=AL.add)
        nc.vector.reciprocal(s, s)
        e_cl_ij = edge[:].unsqueeze(1).broadcast_to([128, 4, K * K])
        nc.vector.tensor_tensor(v_cl_ij, v_cl_ij, e_cl_ij, AL.mult)
        v_ij_cl = maskT[:, ti, :].rearrange("p (ij cl) -> p ij cl", cl=4)
        r_b = rsum[:, ti, :].unsqueeze(1).broadcast_to([128, K * K, 4])
        nc.vector.tensor_tensor(v_ij_cl, v_ij_cl, r_b, AL.mult)

    # interleave convs and mask tiles so the reassembly can start early
    conv1(0)
    for nt in range(8):
        if nt + 1 < 8:
            conv1(nt + 1)
        conv2(nt)
        for sub in range(4):
            mask_tile(nt * 4 + sub)

    # ---- main reassembly ----

    win_cache = {}
    for ti in range(NT):
        h0 = ti * 2
        wins = {}
        for i in range(K):
            for j in range(K):
                if j == 2:
                    continue
                key = (h0 + i + 2, j - 2)
                if key not in win_cache:
                    R0 = key[0] * 64 + key[1]
                    q, sig = R0 // 128, R0 % 128
                    ps = win_ps.tile([128, C], fp32, tag="winps")
                    nc.tensor.matmul(ps[:], shm[:, TAU_IDX[sig], :], xT[:, q, :],
                                     start=True, stop=(sig == 0))
                    if sig != 0:
                        nc.tensor.matmul(ps[:], shm[:, TAU_IDX[sig - 128], :], xT[:, q + 1, :],
                                         start=False, stop=True)
                    w_sb = work.tile([128, C], fp32, tag="win")
                    nc.scalar.copy(out=w_sb[:], in_=ps[:])
                    win_cache[key] = w_sb
                wins[i * K + j] = win_cache[key]
        # retire windows no longer needed (keep pool pressure bounded)
        win_cache = {k: v for k, v in win_cache.items() if k[0] >= h0 + 3}
        stg_tiles = {(di, ch): stagep.tile([96, 2, S * W], fp32, name=f"stg{di}_{ch}", tag=f"stg{di}_{ch}")
                     for di in range(2) for ch in range(2)}
        # All j==2 taps run on PE as mask-weighted (shifted-)diagonal matmuls
        # accumulated into the output transpose PSUM group; GpSimd
        # affine_select builds the diagonals. Even-row taps (2,12,22) are
        # shift-free; odd-row taps (7,17) straddle two xT blocks and use
        # +-64-partition-shifted mask columns (produced on PE).
        PE_TAPS = (2, 12, 22)
        PE_TAPS_ODD = (7, 17)
        cps_dn = mt_ps.tile([128, E], fp32, name="cpsdn", tag="mt")
        nc.tensor.matmul(cps_dn[:], shm[:, TAU_IDX[-64], :], maskT[:, ti, :],
                         start=True, stop=True)
        colsdn = swp.tile([128, E], fp32, name="colsdn", tag="colsh")
        nc.scalar.copy(out=colsdn[:], in_=cps_dn[:])
        cps_up = mt_ps.tile([128, E], fp32, name="cpsup", tag="mt")
        nc.tensor.matmul(cps_up[:], shm[:, TAU_IDX[64], :], maskT[:, ti, :],
                         start=True, stop=True)
        colsup = swp.tile([128, E], fp32, name="colsup", tag="colsh")
        nc.scalar.copy(out=colsup[:], in_=cps_up[:])
        for cl in range(4):
            acc = accp.tile([128, C], fp32, tag="acc")
            first = True
            for ij in range(K * K):
                if ij in PE_TAPS or ij in PE_TAPS_ODD:
                    continue
                col = maskT[:, ti, ij * 4 + cl:ij * 4 + cl + 1]
                if first:
                    nc.vector.tensor_scalar(acc[:], wins[ij][:], col, None, AL.mult)
                    first = False
                else:
                    nc.vector.scalar_tensor_tensor(acc[:], wins[ij][:], col, acc[:],
                                                   AL.mult, AL.add)
            sws = []
            for ij in PE_TAPS:
                col = maskT[:, ti, ij * 4 + cl:ij * 4 + cl + 1]
                sw = swp.tile([128, 128], fp32, name=f"sw{cl}_{ij}", tag="sw")
                nc.gpsimd.affine_select(
                    out=sw[:], in_=col.broadcast_to([128, 128]),
                    compare_op=AL.is_equal, fill=0.0, base=0,
                    channel_multiplier=1, pattern=[[-1, 128]])
                sws.append(sw)
            sws_odd = []
            for ij in PE_TAPS_ODD:
                ch = ij * 4 + cl
                swa = swp.tile([128, 128], fp32, name=f"swa{cl}_{ij}", tag="sw")
                nc.gpsimd.affine_select(
                    out=swa[:], in_=colsdn[:, ch:ch + 1].broadcast_to([128, 128]),
                    compare_op=AL.is_equal, fill=0.0, base=-64,
                    channel_multiplier=1, pattern=[[-1, 128]])
                swb = swp.tile([128, 128], fp32, name=f"swb{cl}_{ij}", tag="sw")
                nc.gpsimd.affine_select(
                    out=swb[:], in_=colsup[:, ch:ch + 1].broadcast_to([128, 128]),
                    compare_op=AL.is_equal, fill=0.0, base=64,
                    channel_multiplier=1, pattern=[[-1, 128]])
                sws_odd.append((swa, swb))
            # transpose acc -> [c, pix] and stage
            di, dj = cl // 2, cl % 2
            for ch in range(2):
                c0 = ch * 96
                pt = out_ps.tile([96, 128], fp32, tag="ot")
                nc.tensor.matmul(pt[:], acc[:, c0:c0 + 96], ident,
                                 is_transpose=True, start=True, stop=False,
                                 skip_group_check=True)
                for k, ij in enumerate(PE_TAPS):
                    i = ij // K
                    q = (h0 + i + 2) * 64 // 128
                    nc.tensor.matmul(pt[:], xT[:, q, c0:c0 + 96], sws[k][:],
                                     start=False, stop=False,
                                     skip_group_check=True)
                for k, ij in enumerate(PE_TAPS_ODD):
                    i = ij // K
                    q = (h0 + i + 2) * 64 // 128  # window spans blocks (q, q+1)
                    swa, swb = sws_odd[k]
                    nc.tensor.matmul(pt[:], xT[:, q, c0:c0 + 96], swa[:],
                                     start=False, stop=False,
                                     skip_group_check=True)
                    nc.tensor.matmul(pt[:], xT[:, q + 1, c0:c0 + 96], swb[:],
                                     start=False, stop=(k == 1),
                                     skip_group_check=True)
                stg = stg_tiles[(di, ch)]
                dst = stg[:].rearrange("c h (w t) -> c h w t", t=2)[:, :, :, dj]
                nc.scalar.copy(out=dst, in_=pt[:].rearrange("c (h w) -> c h w", h=2))
                if dj == 1:
                    nc.sync.dma_start(
                        out=out_d[c0:c0 + 96, h0:h0 + 2, di, :],
                        in_=stg[:])
    es.pop_all().close()


def _host_prep(inputs):
    def fold(w, g, b, m, v):
        s = g / np.sqrt(v + EPS)
        return (w * s[:, None, None, None]).astype(np.float32), (b - m * s).astype(np.float32)

    comp_w_eff, comp_b_eff = fold(inputs["comp_w"], inputs["comp_g"], inputs["comp_b"],
                                  inputs["comp_m"], inputs["comp_v"])
    enc_w_eff, enc_b_eff = fold(inputs["enc_w"], inputs["enc_g"], inputs["enc_b"],
                                inputs["enc_m"], inputs["enc_v"])
    cw = np.ascontiguousarray(comp_w_eff[:, :, 0, 0].T)          # [192, 64]
    cb = comp_b_eff.reshape(Cm, 1)
    ew = np.concatenate([enc_w_eff[:, :, dy, dx].T
                         for dy in range(3) for dx in range(3)], axis=1)  # [64, 900]
    ew = np.ascontiguousarray(ew)
    eb = enc_b_eff.reshape(E, 1)
    wv = np.arange(128) % 64
    edge = np.zeros((128, K * K), np.float32)
    for j in range(K):
        ok = (wv + j - 2 >= 0) & (wv + j - 2 < W)
        for i in range(K):
            edge[:, i * K + j] = ok
    shm = np.zeros((128, len(TAUS), 128), np.float32)
    for t, i in TAU_IDX.items():
        shm[:, i, :] = np.eye(128, dtype=np.float32, k=-t)
    shm = shm.reshape(128, len(TAUS) * 128)
    return dict(cw=cw, cb=cb, ew=ew, eb=eb, edge=edge, shm=shm)


def kernel(**inputs):
    from concourse.bass_utils import run_bass_kernel_spmd

    inputs = {k: np.asarray(v, dtype=np.float32) for k, v in inputs.items()}
    w = _host_prep(inputs)
    if "nc" not in _prog_cache:
        _prog_cache["nc"] = _build_program()
    nc = _prog_cache["nc"]
    x = inputs["x"]
    in_maps = [dict(x=np.ascontiguousarray(x[b].reshape(C, H * W)), **w) for b in range(B)]
    res = run_bass_kernel_spmd(nc, in_maps, list(range(B)))
    out = np.stack([res.results[b]["out"].reshape(C, 2 * H, 2 * W) for b in range(B)])
    return out



# revision 3
# speedup vs baseline: 4.8589x; 4.8589x over previous
# CARAFE (content-aware reassembly) Trainium2 Bass kernel, v2.
# Strategy: data-parallel over batch (8 items -> 8 NeuronCores). Per core:
#   - 1x1 compressor conv + folded BN + SiLU entirely via PE fp32r matmuls
#     (N=512 -> 1 cyc/row) + one ACT Silu per tile.
#   - 3x3 encoder conv as 9 accumulating fp32r matmuls + ACT Exp -> e_sb
#     [100ch, pix] channel-major exp(mask) tensor.
#   - Reassembly on PE in bf16: for each 2-row pixel tile and subpixel-class
#     pair, out[c, p] = sum_i xT_block_i^T @ W_i where W_i are [128, 256]
#     banded matrices holding softmax-normalized mask values on diagonals
#     j-2 in {-2..2}. W is built by ONE gpsimd local_scatter per (tile,
#     class-pair) from data produced with zero partition-shifts:
#     PE-transposing column-shifted slices of e_sb yields all shifted mask
#     columns; constant int16 scatter indices encode tap geometry and edge
#     clipping (idx=-1 drops out-of-image taps, matching x zero-padding).
#   - Softmax normalization: DVE reduce over the transposed (shifted) mask
#     + reciprocal, folded into the scatter data via one strided
#     tensor_tensor (custom APs).
#   - Output accumulated in PSUM [c, (di, h, 2w+dj)], evicted (ACT/DVE) and
#     DMA'd in channel-major order.
import sys
import numpy as np

for _p in ("/opt/trn_rl_repo",):
    if _p not in sys.path:
        sys.path.insert(0, _p)

B, C, Cm, E = 8, 192, 64, 100
H = W = 64
K, S = 5, 2
EPS = 1e-3
NT = 32  # 2-row pixel tiles

_prog_cache = {}


def _build_program(num_devices=8):
    import concourse.mybir as mybir
    import concourse.tile as tile
    from concourse import bacc
    from contextlib import ExitStack

    fp32 = mybir.dt.float32
    nc = bacc.Bacc("TRN2", target_bir_lowering=False, num_devices=num_devices)

    x_d = nc.dram_tensor("x", [C, H * W], fp32, kind="ExternalInput").ap()
    cw_d = nc.dram_tensor("cw", [C, Cm], fp32, kind="ExternalInput").ap()
    cb_d = nc.dram_tensor("cb", [Cm, 1], fp32, kind="ExternalInput").ap()
    ew_d = nc.dram_tensor("ew", [Cm, 9 * E], fp32, kind="ExternalInput").ap()
    eb_d = nc.dram_tensor("eb", [E, 1], fp32, kind="ExternalInput").ap()
    id_d = nc.dram_tensor("ident", [128, 128], fp32, kind="ExternalInput").ap()
    idx_d = nc.dram_tensor("idx", [128, 52], mybir.dt.int16, kind="ExternalInput").ap()
    out_d = nc.dram_tensor("out", [C, H, 2, 2 * W], fp32, kind="ExternalOutput").ap()

    es = ExitStack()
    with tile.TileContext(nc) as tc:
        with es:
            _body(es, tc, nc, mybir,
                  x_d, cw_d, cb_d, ew_d, eb_d, id_d, idx_d, out_d)
    nc.compile()
    return nc


def _body(es, tc, nc, mybir, x_d, cw_d, cb_d, ew_d, eb_d, id_d, idx_d, out_d):
    from concourse.ap import AP
    from concourse import library_config

    fp32 = mybir.dt.float32
    f32r = mybir.dt.float32r
    bf16 = mybir.dt.bfloat16
    AL = mybir.AluOpType
    AF = mybir.ActivationFunctionType

    consts = es.enter_context(tc.tile_pool(name="consts", bufs=1))
    big = es.enter_context(tc.tile_pool(name="big", bufs=1))

    cw0 = consts.tile([128, Cm], fp32, tag="cw0")
    cw1 = consts.tile([64, Cm], fp32, tag="cw1")
    cb = consts.tile([Cm, 1], fp32, tag="cb")
    ew = consts.tile([Cm, 9 * E], fp32, tag="ew")
    eb = consts.tile([E, 1], fp32, tag="eb")
    ident = consts.tile([128, 128], fp32, tag="ident")
    idxt = consts.tile([128, 52], mybir.dt.int16, tag="idxt")
    zeroT = consts.tile([128, 192], bf16, tag="zeroT")

    x0 = big.tile([128, H * W], fp32, tag="x0")
    x1 = big.tile([64, H * W], fp32, tag="x1")
    t_pad = big.tile([Cm, 66 * 66], fp32, tag="tpad")
    e_sb = big.tile([E, H * W + 4], fp32, tag="esb")
    xTe = big.tile([128, 36, C], bf16, tag="xTe")
    xTo = big.tile([128, 36, C], bf16, tag="xTo")

    # PSUM pools (8 banks total: 2+1+2+2+1)
    t5ps = es.enter_context(tc.tile_pool(name="t5ps", bufs=2, space="PSUM"))
    convps = es.enter_context(tc.tile_pool(name="convps", bufs=1, space="PSUM"))
    out0ps = es.enter_context(tc.tile_pool(name="out0ps", bufs=2, space="PSUM"))
    out1ps = es.enter_context(tc.tile_pool(name="out1ps", bufs=2, space="PSUM"))
    xtps = es.enter_context(tc.tile_pool(name="xtps", bufs=1, space="PSUM"))

    matsp = es.enter_context(tc.tile_pool(name="matsp", bufs=3))
    rsump = es.enter_context(tc.tile_pool(name="rsump", bufs=3))
    rinvp = es.enter_context(tc.tile_pool(name="rinvp", bufs=3))
    datap = es.enter_context(tc.tile_pool(name="datap", bufs=3))
    wp = es.enter_context(tc.tile_pool(name="wp", bufs=4))
    stg0p = es.enter_context(tc.tile_pool(name="stg0p", bufs=2))
    stg1p = es.enter_context(tc.tile_pool(name="stg1p", bufs=2))

    R = lambda ap: ap.bitcast(f32r)

    nc.gpsimd.load_library(library_config.local_scatter)

    # ---- input DMAs ----
    nc.sync.dma_start(out=cw0[:], in_=cw_d[0:128, :])
    nc.sync.dma_start(out=cw1[:], in_=cw_d[128:192, :])
    nc.sync.dma_start(out=cb[:], in_=cb_d)
    nc.sync.dma_start(out=ew[:], in_=ew_d)
    nc.sync.dma_start(out=eb[:], in_=eb_d)
    nc.sync.dma_start(out=ident[:], in_=id_d)
    nc.sync.dma_start(out=idxt[:], in_=idx_d)
    for ck in range(4):
        c0 = ck * 1024
        nc.sync.dma_start(out=x0[:, c0:c0 + 1024], in_=x_d[0:128, c0:c0 + 1024])
        nc.sync.dma_start(out=x1[:, c0:c0 + 1024], in_=x_d[128:192, c0:c0 + 1024])

    # ---- border memsets ----
    nc.gpsimd.memset(zeroT[:], 0.0)
    tp3 = t_pad[:].rearrange("c (r z) -> c r z", z=66)
    nc.gpsimd.memset(tp3[:, 0, :], 0.0)
    nc.gpsimd.memset(tp3[:, 65, :], 0.0)
    nc.gpsimd.memset(tp3[:, 1:65, 0:1], 0.0)
    nc.gpsimd.memset(tp3[:, 1:65, 65:66], 0.0)
    nc.gpsimd.memset(e_sb[:, 0:2], 1.0)
    nc.gpsimd.memset(e_sb[:, H * W + 2:H * W + 4], 1.0)
    # zero x-row border blocks of xT (CARAFE zero padding outside the image)
    for t, b in ((xTe, 1), (xTe, 34), (xTo, 1), (xTo, 33)):
        nc.vector.tensor_copy(t[:, b, :], zeroT[:])

    # ---- helpers ----
    def conv1(nt):
        n0 = nt * 512
        ps = convps.tile([E, 512], fp32, name="c1ps", tag="conv")
        nc.tensor.matmul(ps[0:Cm, :], R(cw0[:]), R(x0[:, n0:n0 + 512]),
                         start=True, stop=False)
        nc.tensor.matmul(ps[0:Cm, :], R(cw1[:]), R(x1[:, n0:n0 + 512]),
                         start=False, stop=True)
        v = tp3[:, nt * 8 + 1: nt * 8 + 9, 1:65]
        nc.scalar.activation(out=v, in_=ps[0:Cm, :].rearrange("c (r z) -> c r z", z=64),
                             func=AF.Silu, bias=cb[:], scale=1.0)

    def conv2(nt):
        r0 = nt * 8
        ps = convps.tile([E, 512], fp32, name="c2ps", tag="conv")
        for tap in range(9):
            dy, dx = tap // 3, tap % 3
            rhs = tp3[:, r0 + dy: r0 + dy + 8, dx: dx + 64]
            nc.tensor.matmul(ps[:], R(ew[:, tap * E:(tap + 1) * E]), R(rhs),
                             start=(tap == 0), stop=(tap == 8))
        nc.scalar.activation(out=e_sb[:, 2 + r0 * 64: 2 + r0 * 64 + 512], in_=ps[:],
                             func=AF.Exp, bias=eb[:], scale=1.0)

    nxt = [0]  # alternate eviction engine for xT blocks

    def _xt_evict(dst, src):
        if nxt[0] % 2 == 0:
            nc.scalar.copy(out=dst, in_=src)
        else:
            nc.vector.tensor_copy(dst, src)
        nxt[0] += 1

    def xte_block(be):
        px0 = 128 * (be - 2)
        pt = xtps.tile([128, C], fp32, name="xtpt", tag="xt")
        nc.tensor.matmul(R(pt[:, 0:128]), R(x0[:, px0:px0 + 128]), R(ident[:]),
                         is_transpose=True, skip_group_check=True)
        nc.tensor.matmul(R(pt[:, 128:192]), R(x1[:, px0:px0 + 128]),
                         R(ident[0:64, 0:64]), is_transpose=True,
                         skip_group_check=True)
        _xt_evict(xTe[:, be, :], pt[:])

    def xto_block(bo):
        px0 = 128 * (bo - 2) + 64
        pt = xtps.tile([128, C], fp32, name="xopt", tag="xt")
        nc.tensor.matmul(R(pt[:, 0:128]), R(x0[:, px0:px0 + 128]), R(ident[:]),
                         is_transpose=True, skip_group_check=True)
        nc.tensor.matmul(R(pt[:, 128:192]), R(x1[:, px0:px0 + 128]),
                         R(ident[0:64, 0:64]), is_transpose=True,
                         skip_group_check=True)
        _xt_evict(xTo[:, bo, :], pt[:])

    def xto_half(bo):
        pt = xtps.tile([128, C], fp32, name="xhpt", tag="xt")
        if bo == 1:  # rows (-1, 0): only upper 64 partitions hold row 0
            cols, prt = slice(0, 64), slice(64, 128)
        else:  # bo == 33: rows (63, 64): lower 64 partitions hold row 63
            cols, prt = slice(4032, 4096), slice(0, 64)
        nc.tensor.matmul(R(pt[prt, 0:128]), R(x0[:, cols]), R(ident[:]),
                         is_transpose=True, skip_group_check=True)
        nc.tensor.matmul(R(pt[prt, 128:192]), R(x1[:, cols]),
                         R(ident[0:64, 0:64]), is_transpose=True,
                         skip_group_check=True)
        _xt_evict(xTo[prt, bo, :], pt[prt, :])

    data_tiles = {}
    w_tiles = {}

    def stageA(ti):
        # 5 shifted transposes of e_sb -> T5 [128, 5, 100] (psum), then
        # normalize into bf16 scatter data [128, 4cl, 26].
        p0 = 128 * ti
        t5 = t5ps.tile([128, 500], fp32, name="t5", tag="t5")
        for d in range(5):
            s = p0 + 4 - d
            nc.tensor.matmul(R(t5[:, d * 100:(d + 1) * 100]),
                             R(e_sb[:, s:s + 128]), R(ident[0:E, 0:E]),
                             is_transpose=True, skip_group_check=True)
        mats = matsp.tile([128, 500], bf16, name="mats", tag="mats")
        nc.scalar.copy(out=mats[:], in_=t5[:])
        rsum = rsump.tile([128, 20], fp32, name="rsum", tag="rsum")
        red_in = AP(t5[:].tensor, 0, [[500, 128], [100, 5], [1, 4], [4, 25]])
        nc.vector.tensor_reduce(out=rsum[:].rearrange("q (d c) -> q d c", c=4),
                                in_=red_in, axis=mybir.AxisListType.X, op=AL.add)
        rinv = rinvp.tile([128, 20], fp32, name="rinv", tag="rinv")
        nc.vector.reciprocal(rinv[:], rsum[:])
        data = datap.tile([128, 4, 26], bf16, name="data", tag="data")
        # data[q, cl, i*5+j] = mats[q, 104j + 20i + cl] * rinv[q, 4j + cl]
        in0 = AP(mats[:].tensor, 0, [[500, 128], [1, 4], [20, 5], [104, 5]])
        in1 = AP(rinv[:].tensor, 0, [[20, 128], [1, 4], [0, 5], [4, 5]])
        outv = AP(data[:].tensor, 0, [[104, 128], [26, 4], [5, 5], [1, 5]])
        nc.vector.tensor_tensor(outv, in0, in1, AL.mult)
        data_tiles[ti] = data

    def stageB(ti):
        data = data_tiles.pop(ti)
        for di in range(2):
            w = wp.tile([128, 5, 256], bf16, name=f"w{di}", tag=f"w{di}")
            nc.gpsimd.local_scatter(
                out_ap=w[:].rearrange("q a b -> q (a b)"),
                data_ap=data[:, 2 * di:2 * di + 2, :].rearrange("q a b -> q (a b)"),
                idxs_ap=idxt[:],
                channels=128, num_elems=1280, num_idxs=52)
            w_tiles[(ti, di)] = w

    def stageC(ti):
        for ck, (c0, cp, psp, stgp, ev) in enumerate(
                ((0, 128, out0ps, stg0p, "act"), (128, 64, out1ps, stg1p, "dve"))):
            ps = psp.tile([cp, 512], fp32, name=f"ops{ck}", tag=f"o{ck}")
            for di in range(2):
                w = w_tiles[(ti, di)]
                # psum columns in output row-major order (h, di, w'): pair di
                # writes the strided view [cp, 2h, 128w'] at offset di*128.
                ov = AP(ps[:].tensor, di * 128, [[512, cp], [256, 2], [1, 128]])
                for i in range(5):
                    if i % 2 == 0:
                        blk = xTe[:, ti + 1 + i // 2, c0:c0 + cp]
                    else:
                        blk = xTo[:, ti + 1 + (i - 1) // 2, c0:c0 + cp]
                    nc.tensor.matmul(ov, blk, w[:, i, :],
                                     start=(i == 0), stop=(i == 4),
                                     skip_group_check=True)
            stg = stgp.tile([cp, 512], fp32, name=f"stg{ck}", tag=f"s{ck}")
            if ev == "act":
                nc.scalar.copy(out=stg[:], in_=ps[:])
            else:
                nc.vector.tensor_copy(stg[:], ps[:])
            nc.sync.dma_start(
                out=out_d[c0:c0 + cp, 2 * ti:2 * ti + 2, :, :].rearrange(
                    "c h a w -> c (h a w)"),
                in_=stg[:])
        del w_tiles[(ti, 0)], w_tiles[(ti, 1)]

    # ---- prologue ----
    # conv1 over the whole image (single Silu act-table load), interleaved
    # with the first xT transposes.
    for nt in range(8):
        conv1(nt)
        if nt == 0:
            xte_block(2)
        elif nt == 1:
            xte_block(3)
        elif nt == 2:
            xto_half(1)
        elif nt == 3:
            xto_block(2)
    conv2(0)
    conv2(1)
    stageA(0)
    stageA(1)
    stageB(0)

    # ---- main pipelined loop ----
    for it in range(NT):
        if it + 4 <= 33:
            xte_block(it + 4)
        if it + 3 <= 32:
            xto_block(it + 3)
        if it == 30:
            xto_half(33)
        if it % 4 == 2 and it // 4 + 2 <= 7:
            conv2(it // 4 + 2)
        if it + 2 < NT:
            stageA(it + 2)
        if it + 1 < NT:
            stageB(it + 1)
        stageC(it)
    es.pop_all().close()


def _host_prep(inputs):
    def fold(w, g, b, m, v):
        s = g / np.sqrt(v + EPS)
        return (w * s[:, None, None, None]).astype(np.float32), (b - m * s).astype(np.float32)

    comp_w_eff, comp_b_eff = fold(inputs["comp_w"], inputs["comp_g"], inputs["comp_b"],
                                  inputs["comp_m"], inputs["comp_v"])
    enc_w_eff, enc_b_eff = fold(inputs["enc_w"], inputs["enc_g"], inputs["enc_b"],
                                inputs["enc_m"], inputs["enc_v"])
    cw = np.ascontiguousarray(comp_w_eff[:, :, 0, 0].T)          # [192, 64]
    cb = comp_b_eff.reshape(Cm, 1)
    ew = np.concatenate([enc_w_eff[:, :, dy, dx].T
                         for dy in range(3) for dx in range(3)], axis=1)  # [64, 900]
    ew = np.ascontiguousarray(ew)
    eb = enc_b_eff.reshape(E, 1)
    ident = np.eye(128, dtype=np.float32)
    idx = np.full((128, 52), -1, np.int16)
    for q in range(128):
        hq, wq = q // 64, q % 64
        for dj in range(2):
            for i in range(K):
                for j in range(K):
                    wt = wq - (j - 2)
                    if 0 <= wt < W:
                        idx[q, dj * 26 + i * 5 + j] = i * 256 + hq * 128 + 2 * wt + dj
    return dict(cw=cw, cb=cb, ew=ew, eb=eb, ident=ident, idx=idx)


def kernel(**inputs):
    from concourse.bass_utils import run_bass_kernel_spmd

    inputs = {k: np.asarray(v, dtype=np.float32) if np.asarray(v).dtype != np.int16
              else np.asarray(v) for k, v in inputs.items()}
    w = _host_prep(inputs)
    if "nc" not in _prog_cache:
        _prog_cache["nc"] = _build_program()
    nc = _prog_cache["nc"]
    x = inputs["x"]
    in_maps = [dict(x=np.ascontiguousarray(x[b].reshape(C, H * W)), **w) for b in range(B)]
    res = run_bass_kernel_spmd(nc, in_maps, list(range(B)))
    out = np.stack([res.results[b]["out"].reshape(C, 2 * H, 2 * W) for b in range(B)])
    return out


# revision 7
# speedup vs baseline: 5.7343x; 1.1802x over previous
# CARAFE (content-aware reassembly) Trainium2 Bass kernel, v2.
# Strategy: data-parallel over batch (8 items -> 8 NeuronCores). Per core:
#   - 1x1 compressor conv + folded BN + SiLU entirely via PE fp32r matmuls
#     (N=512 -> 1 cyc/row) + one ACT Silu per tile.
#   - 3x3 encoder conv as 9 accumulating fp32r matmuls + ACT Exp -> e_sb
#     [100ch, pix] channel-major exp(mask) tensor.
#   - Reassembly on PE in bf16: for each 2-row pixel tile and subpixel-class
#     pair, out[c, p] = sum_i xT_block_i^T @ W_i where W_i are [128, 256]
#     banded matrices holding softmax-normalized mask values on diagonals
#     j-2 in {-2..2}. W is built by ONE gpsimd local_scatter per (tile,
#     class-pair) from data produced with zero partition-shifts:
#     PE-transposing column-shifted slices of e_sb yields all shifted mask
#     columns; constant int16 scatter indices encode tap geometry and edge
#     clipping (idx=-1 drops out-of-image taps, matching x zero-padding).
#   - Softmax normalization: DVE reduce over the transposed (shifted) mask
#     + reciprocal, folded into the scatter data via one strided
#     tensor_tensor (custom APs).
#   - Output accumulated in PSUM [c, (di, h, 2w+dj)], evicted (ACT/DVE) and
#     DMA'd in channel-major order.
import sys
import numpy as np

for _p in ("/opt/trn_rl_repo",):
    if _p not in sys.path:
        sys.path.insert(0, _p)

B, C, Cm, E = 8, 192, 64, 100
H = W = 64
K, S = 5, 2
EPS = 1e-3
NT = 32  # 2-row pixel tiles

_prog_cache = {}


def _build_program(num_devices=8):
    import concourse.mybir as mybir
    import concourse.tile as tile
    from concourse import bacc
    from contextlib import ExitStack

    fp32 = mybir.dt.float32
    nc = bacc.Bacc("TRN2", target_bir_lowering=False, num_devices=num_devices)

    x_d = nc.dram_tensor("x", [C, H * W], fp32, kind="ExternalInput").ap()
    cw_d = nc.dram_tensor("cw", [C, Cm], fp32, kind="ExternalInput").ap()
    cb_d = nc.dram_tensor("cb", [Cm, 1], fp32, kind="ExternalInput").ap()
    ew_d = nc.dram_tensor("ew", [Cm, 9 * E], fp32, kind="ExternalInput").ap()
    eb_d = nc.dram_tensor("eb", [E, 1], fp32, kind="ExternalInput").ap()
    id_d = nc.dram_tensor("ident", [128, 128], fp32, kind="ExternalInput").ap()
    idx_d = nc.dram_tensor("idx", [128, 104], mybir.dt.int16, kind="ExternalInput").ap()
    out_d = nc.dram_tensor("out", [C, H, 2, 2 * W], fp32, kind="ExternalOutput").ap()

    es = ExitStack()
    with tile.TileContext(nc) as tc:
        with es:
            _body(es, tc, nc, mybir,
                  x_d, cw_d, cb_d, ew_d, eb_d, id_d, idx_d, out_d)
    nc.compile()
    return nc


def _body(es, tc, nc, mybir, x_d, cw_d, cb_d, ew_d, eb_d, id_d, idx_d, out_d):
    from concourse.ap import AP
    from concourse import library_config

    fp32 = mybir.dt.float32
    f32r = mybir.dt.float32r
    bf16 = mybir.dt.bfloat16
    AL = mybir.AluOpType
    AF = mybir.ActivationFunctionType

    consts = es.enter_context(tc.tile_pool(name="consts", bufs=1))
    big = es.enter_context(tc.tile_pool(name="big", bufs=1))

    cw0 = consts.tile([128, Cm], fp32, tag="cw0")
    cw1 = consts.tile([64, Cm], fp32, tag="cw1")
    cb = consts.tile([Cm, 1], fp32, tag="cb")
    ew = consts.tile([Cm, 9 * E], fp32, tag="ew")
    eb = consts.tile([E, 1], fp32, tag="eb")
    ident = consts.tile([128, 128], fp32, tag="ident")
    idxt = consts.tile([128, 104], mybir.dt.int16, tag="idxt")
    zeroT = consts.tile([128, 192], bf16, tag="zeroT")

    x0 = big.tile([128, H * W], fp32, tag="x0")
    x1 = big.tile([64, H * W], fp32, tag="x1")
    t_pad = big.tile([Cm, 66 * 66], fp32, tag="tpad")
    e_sb = big.tile([E, H * W + 4], fp32, tag="esb")
    xTe = big.tile([128, 36, C], bf16, tag="xTe")
    xTo = big.tile([128, 36, C], bf16, tag="xTo")

    # PSUM pools (8 banks total: 2+1+2+2+1)
    t5ps = es.enter_context(tc.tile_pool(name="t5ps", bufs=2, space="PSUM"))
    convps = es.enter_context(tc.tile_pool(name="convps", bufs=1, space="PSUM"))
    out0ps = es.enter_context(tc.tile_pool(name="out0ps", bufs=2, space="PSUM"))
    out1ps = es.enter_context(tc.tile_pool(name="out1ps", bufs=2, space="PSUM"))
    xtps = es.enter_context(tc.tile_pool(name="xtps", bufs=1, space="PSUM"))

    matsp = es.enter_context(tc.tile_pool(name="matsp", bufs=3))
    rsump = es.enter_context(tc.tile_pool(name="rsump", bufs=3))
    rinvp = es.enter_context(tc.tile_pool(name="rinvp", bufs=3))
    datap = es.enter_context(tc.tile_pool(name="datap", bufs=3))
    wp = es.enter_context(tc.tile_pool(name="wp", bufs=3))
    stg0p = es.enter_context(tc.tile_pool(name="stg0p", bufs=2))
    stg1p = es.enter_context(tc.tile_pool(name="stg1p", bufs=2))

    R = lambda ap: ap.bitcast(f32r)

    nc.gpsimd.load_library(library_config.local_scatter)

    # ---- input DMAs ----
    nc.sync.dma_start(out=cw0[:], in_=cw_d[0:128, :])
    nc.sync.dma_start(out=cw1[:], in_=cw_d[128:192, :])
    nc.sync.dma_start(out=cb[:], in_=cb_d)
    nc.sync.dma_start(out=ew[:], in_=ew_d)
    nc.sync.dma_start(out=eb[:], in_=eb_d)
    nc.sync.dma_start(out=ident[:], in_=id_d)
    nc.sync.dma_start(out=idxt[:], in_=idx_d)
    for ck in range(4):
        c0 = ck * 1024
        nc.sync.dma_start(out=x0[:, c0:c0 + 1024], in_=x_d[0:128, c0:c0 + 1024])
        nc.sync.dma_start(out=x1[:, c0:c0 + 1024], in_=x_d[128:192, c0:c0 + 1024])

    # ---- border memsets ----
    nc.gpsimd.memset(zeroT[:], 0.0)
    tp3 = t_pad[:].rearrange("c (r z) -> c r z", z=66)
    nc.gpsimd.memset(tp3[:, 0, :], 0.0)
    nc.gpsimd.memset(tp3[:, 65, :], 0.0)
    nc.gpsimd.memset(tp3[:, 1:65, 0:1], 0.0)
    nc.gpsimd.memset(tp3[:, 1:65, 65:66], 0.0)
    nc.gpsimd.memset(e_sb[:, 0:2], 1.0)
    nc.gpsimd.memset(e_sb[:, H * W + 2:H * W + 4], 1.0)
    # zero x-row border blocks of xT (CARAFE zero padding outside the image)
    for t, b in ((xTe, 1), (xTe, 34), (xTo, 1), (xTo, 33)):
        nc.vector.tensor_copy(t[:, b, :], zeroT[:])

    # ---- helpers ----
    def conv1(nt):
        n0 = nt * 512
        ps = convps.tile([E, 512], fp32, name="c1ps", tag="conv")
        nc.tensor.matmul(ps[0:Cm, :], R(cw0[:]), R(x0[:, n0:n0 + 512]),
                         start=True, stop=False)
        nc.tensor.matmul(ps[0:Cm, :], R(cw1[:]), R(x1[:, n0:n0 + 512]),
                         start=False, stop=True)
        v = tp3[:, nt * 8 + 1: nt * 8 + 9, 1:65]
        nc.scalar.activation(out=v, in_=ps[0:Cm, :].rearrange("c (r z) -> c r z", z=64),
                             func=AF.Silu, bias=cb[:], scale=1.0)

    def conv2(nt):
        r0 = nt * 8
        ps = convps.tile([E, 512], fp32, name="c2ps", tag="conv")
        for tap in range(9):
            dy, dx = tap // 3, tap % 3
            rhs = tp3[:, r0 + dy: r0 + dy + 8, dx: dx + 64]
            nc.tensor.matmul(ps[:], R(ew[:, tap * E:(tap + 1) * E]), R(rhs),
                             start=(tap == 0), stop=(tap == 8))
        nc.scalar.activation(out=e_sb[:, 2 + r0 * 64: 2 + r0 * 64 + 512], in_=ps[:],
                             func=AF.Exp, bias=eb[:], scale=1.0)

    nxt = [0]  # alternate eviction engine for xT blocks

    def _xt_evict(dst, src):
        if nxt[0] % 2 == 0:
            nc.scalar.copy(out=dst, in_=src)
        else:
            nc.vector.tensor_copy(dst, src)
        nxt[0] += 1

    def xte_block(be):
        px0 = 128 * (be - 2)
        pt = xtps.tile([128, C], fp32, name="xtpt", tag="xt")
        nc.tensor.matmul(R(pt[:, 0:128]), R(x0[:, px0:px0 + 128]), R(ident[:]),
                         is_transpose=True, skip_group_check=True)
        nc.tensor.matmul(R(pt[:, 128:192]), R(x1[:, px0:px0 + 128]),
                         R(ident[0:64, 0:64]), is_transpose=True,
                         skip_group_check=True)
        _xt_evict(xTe[:, be, :], pt[:])

    def xto_block(bo):
        px0 = 128 * (bo - 2) + 64
        pt = xtps.tile([128, C], fp32, name="xopt", tag="xt")
        nc.tensor.matmul(R(pt[:, 0:128]), R(x0[:, px0:px0 + 128]), R(ident[:]),
                         is_transpose=True, skip_group_check=True)
        nc.tensor.matmul(R(pt[:, 128:192]), R(x1[:, px0:px0 + 128]),
                         R(ident[0:64, 0:64]), is_transpose=True,
                         skip_group_check=True)
        _xt_evict(xTo[:, bo, :], pt[:])

    def xto_half(bo):
        pt = xtps.tile([128, C], fp32, name="xhpt", tag="xt")
        if bo == 1:  # rows (-1, 0): only upper 64 partitions hold row 0
            cols, prt = slice(0, 64), slice(64, 128)
        else:  # bo == 33: rows (63, 64): lower 64 partitions hold row 63
            cols, prt = slice(4032, 4096), slice(0, 64)
        nc.tensor.matmul(R(pt[prt, 0:128]), R(x0[:, cols]), R(ident[:]),
                         is_transpose=True, skip_group_check=True)
        nc.tensor.matmul(R(pt[prt, 128:192]), R(x1[:, cols]),
                         R(ident[0:64, 0:64]), is_transpose=True,
                         skip_group_check=True)
        _xt_evict(xTo[prt, bo, :], pt[prt, :])

    data_tiles = {}
    w_tiles = {}

    def stageA(ti):
        # 5 shifted transposes of e_sb -> T5 [128, 5, 100] (psum), then
        # normalize into bf16 scatter data [128, 4cl, 26].
        p0 = 128 * ti
        t5 = t5ps.tile([128, 500], fp32, name="t5", tag="t5")
        for d in range(5):
            s = p0 + 4 - d
            nc.tensor.matmul(R(t5[:, d * 100:(d + 1) * 100]),
                             R(e_sb[:, s:s + 128]), R(ident[0:E, 0:E]),
                             is_transpose=True, skip_group_check=True)
        mats = matsp.tile([128, 500], bf16, name="mats", tag="mats")
        nc.scalar.copy(out=mats[:], in_=t5[:])
        rsum = rsump.tile([128, 20], fp32, name="rsum", tag="rsum")
        red_in = AP(t5[:].tensor, 0, [[500, 128], [100, 5], [1, 4], [4, 25]])
        nc.vector.tensor_reduce(out=rsum[:].rearrange("q (d c) -> q d c", c=4),
                                in_=red_in, axis=mybir.AxisListType.X, op=AL.add)
        rinv = rinvp.tile([128, 20], fp32, name="rinv", tag="rinv")
        nc.vector.reciprocal(rinv[:], rsum[:])
        data = datap.tile([128, 4, 26], bf16, name="data", tag="data")
        # data[q, cl, i*5+j] = mats[q, 104j + 20i + cl] * rinv[q, 4j + cl]
        in0 = AP(mats[:].tensor, 0, [[500, 128], [1, 4], [20, 5], [104, 5]])
        in1 = AP(rinv[:].tensor, 0, [[20, 128], [1, 4], [0, 5], [4, 5]])
        outv = AP(data[:].tensor, 0, [[104, 128], [26, 4], [5, 5], [1, 5]])
        nc.vector.tensor_tensor(outv, in0, in1, AL.mult)
        data_tiles[ti] = data

    def stageB(ti):
        # One scatter builds the whole tile's W [128, (di, i, 2w+dj)]: the
        # output-row dimension is implicit in the partition halves (q<64 ->
        # out row h0, q>=64 -> h0+1), so no structurally-zero quadrants.
        data = data_tiles.pop(ti)
        w = wp.tile([128, 2, 5, 128], bf16, name="w", tag="w")
        nc.gpsimd.local_scatter(
            out_ap=w[:].rearrange("q a b c -> q (a b c)"),
            data_ap=data[:].rearrange("q a b -> q (a b)"),
            idxs_ap=idxt[:],
            channels=128, num_elems=1280, num_idxs=104)
        w_tiles[ti] = w

    def stageC(ti):
        w = w_tiles[ti]
        for ck, (c0, cp, psp, stgp, ev) in enumerate(
                ((0, 128, out0ps, stg0p, "act"), (128, 64, out1ps, stg1p, "dve"))):
            # psum columns in output row-major order (h, di, w'): K=64
            # matmuls pair partition half hq of the x block with the same
            # half of W (out row h0+hq).
            ps = psp.tile([cp, 512], fp32, name=f"ops{ck}", tag=f"o{ck}")
            for hq, qs in ((0, slice(0, 64)), (1, slice(64, 128))):
                for di in range(2):
                    n0 = hq * 256 + di * 128
                    for i in range(5):
                        if i % 2 == 0:
                            blk = xTe[qs, ti + 1 + i // 2, c0:c0 + cp]
                        else:
                            blk = xTo[qs, ti + 1 + (i - 1) // 2, c0:c0 + cp]
                        nc.tensor.matmul(ps[:, n0:n0 + 128], blk, w[qs, di, i, :],
                                         start=(i == 0), stop=(i == 4),
                                         skip_group_check=True)
            stg = stgp.tile([cp, 512], fp32, name=f"stg{ck}", tag=f"s{ck}")
            if ev == "act":
                nc.scalar.copy(out=stg[:], in_=ps[:])
            else:
                nc.vector.tensor_copy(stg[:], ps[:])
            nc.sync.dma_start(
                out=out_d[c0:c0 + cp, 2 * ti:2 * ti + 2, :, :].rearrange(
                    "c h a w -> c (h a w)"),
                in_=stg[:])
        del w_tiles[ti]

    # ---- prologue ----
    # conv1 over the whole image (single Silu act-table load), interleaved
    # with the first xT transposes.
    for nt in range(8):
        conv1(nt)
        if nt == 0:
            xte_block(2)
        elif nt == 1:
            xte_block(3)
        elif nt == 2:
            xto_half(1)
        elif nt == 3:
            xto_block(2)
    conv2(0)
    conv2(1)
    stageA(0)
    stageA(1)
    stageB(0)

    # ---- main pipelined loop ----
    for it in range(NT):
        if it + 4 <= 33:
            xte_block(it + 4)
        if it + 3 <= 32:
            xto_block(it + 3)
        if it == 30:
            xto_half(33)
        if it % 4 == 2 and it // 4 + 2 <= 7:
            conv2(it // 4 + 2)
        if it + 2 < NT:
            stageA(it + 2)
        if it + 1 < NT:
            stageB(it + 1)
        stageC(it)
    es.pop_all().close()


def _host_prep(inputs):
    def fold(w, g, b, m, v):
        s = g / np.sqrt(v + EPS)
        return (w * s[:, None, None, None]).astype(np.float32), (b - m * s).astype(np.float32)

    comp_w_eff, comp_b_eff = fold(inputs["comp_w"], inputs["comp_g"], inputs["comp_b"],
                                  inputs["comp_m"], inputs["comp_v"])
    enc_w_eff, enc_b_eff = fold(inputs["enc_w"], inputs["enc_g"], inputs["enc_b"],
                                inputs["enc_m"], inputs["enc_v"])
    cw = np.ascontiguousarray(comp_w_eff[:, :, 0, 0].T)          # [192, 64]
    cb = comp_b_eff.reshape(Cm, 1)
    ew = np.concatenate([enc_w_eff[:, :, dy, dx].T
                         for dy in range(3) for dx in range(3)], axis=1)  # [64, 900]
    ew = np.ascontiguousarray(ew)
    eb = enc_b_eff.reshape(E, 1)
    ident = np.eye(128, dtype=np.float32)
    idx = np.full((128, 104), -1, np.int16)
    for q in range(128):
        wq = q % 64
        for cl in range(4):
            di, dj = cl // 2, cl % 2
            for i in range(K):
                for j in range(K):
                    wt = wq - (j - 2)
                    if 0 <= wt < W:
                        idx[q, cl * 26 + i * 5 + j] = di * 640 + i * 128 + 2 * wt + dj
    return dict(cw=cw, cb=cb, ew=ew, eb=eb, ident=ident, idx=idx)


def kernel(**inputs):
    from concourse.bass_utils import run_bass_kernel_spmd

    inputs = {k: np.asarray(v, dtype=np.float32) if np.asarray(v).dtype != np.int16
              else np.asarray(v) for k, v in inputs.items()}
    w = _host_prep(inputs)
    if "nc" not in _prog_cache:
        _prog_cache["nc"] = _build_program()
    nc = _prog_cache["nc"]
    x = inputs["x"]
    in_maps = [dict(x=np.ascontiguousarray(x[b].reshape(C, H * W)), **w) for b in range(B)]
    res = run_bass_kernel_spmd(nc, in_maps, list(range(B)))
    out = np.stack([res.results[b]["out"].reshape(C, 2 * H, 2 * W) for b in range(B)])
    return out


# revision 10
# speedup vs baseline: 6.0535x; 1.0557x over previous
# CARAFE (content-aware reassembly) Trainium2 Bass kernel, v2.
# Strategy: data-parallel over batch (8 items -> 8 NeuronCores). Per core:
#   - 1x1 compressor conv + folded BN + SiLU entirely via PE fp32r matmuls
#     (N=512 -> 1 cyc/row) + one ACT Silu per tile.
#   - 3x3 encoder conv as 9 accumulating fp32r matmuls + ACT Exp -> e_sb
#     [100ch, pix] channel-major exp(mask) tensor.
#   - Reassembly on PE in bf16: for each 2-row pixel tile and subpixel-class
#     pair, out[c, p] = sum_i xT_block_i^T @ W_i where W_i are [128, 256]
#     banded matrices holding softmax-normalized mask values on diagonals
#     j-2 in {-2..2}. W is built by ONE gpsimd local_scatter per (tile,
#     class-pair) from data produced with zero partition-shifts:
#     PE-transposing column-shifted slices of e_sb yields all shifted mask
#     columns; constant int16 scatter indices encode tap geometry and edge
#     clipping (idx=-1 drops out-of-image taps, matching x zero-padding).
#   - Softmax normalization: DVE reduce over the transposed (shifted) mask
#     + reciprocal, folded into the scatter data via one strided
#     tensor_tensor (custom APs).
#   - Output accumulated in PSUM [c, (di, h, 2w+dj)], evicted (ACT/DVE) and
#     DMA'd in channel-major order.
import sys
import numpy as np

for _p in ("/opt/trn_rl_repo",):
    if _p not in sys.path:
        sys.path.insert(0, _p)

B, C, Cm, E = 8, 192, 64, 100
H = W = 64
K, S = 5, 2
EPS = 1e-3
NT = 32  # 2-row pixel tiles

_prog_cache = {}


def _build_program(num_devices=8):
    import concourse.mybir as mybir
    import concourse.tile as tile
    from concourse import bacc
    from contextlib import ExitStack

    fp32 = mybir.dt.float32
    nc = bacc.Bacc("TRN2", target_bir_lowering=False, num_devices=num_devices)

    x_d = nc.dram_tensor("x", [C, H * W], fp32, kind="ExternalInput").ap()
    cw_d = nc.dram_tensor("cw", [C, Cm], fp32, kind="ExternalInput").ap()
    cb_d = nc.dram_tensor("cb", [Cm, 1], fp32, kind="ExternalInput").ap()
    ew_d = nc.dram_tensor("ew", [128, 6 * E], fp32, kind="ExternalInput").ap()
    eb_d = nc.dram_tensor("eb", [E, 1], fp32, kind="ExternalInput").ap()
    id_d = nc.dram_tensor("ident", [128, 128], fp32, kind="ExternalInput").ap()
    idx_d = nc.dram_tensor("idx", [128, 104], mybir.dt.int16, kind="ExternalInput").ap()
    out_d = nc.dram_tensor("out", [C, H, 2, 2 * W], fp32, kind="ExternalOutput").ap()

    es = ExitStack()
    with tile.TileContext(nc) as tc:
        with es:
            _body(es, tc, nc, mybir,
                  x_d, cw_d, cb_d, ew_d, eb_d, id_d, idx_d, out_d)
    nc.compile()
    return nc


def _body(es, tc, nc, mybir, x_d, cw_d, cb_d, ew_d, eb_d, id_d, idx_d, out_d):
    from concourse.ap import AP
    from concourse import library_config

    fp32 = mybir.dt.float32
    f32r = mybir.dt.float32r
    bf16 = mybir.dt.bfloat16
    AL = mybir.AluOpType
    AF = mybir.ActivationFunctionType

    consts = es.enter_context(tc.tile_pool(name="consts", bufs=1))
    big = es.enter_context(tc.tile_pool(name="big", bufs=1))

    cw0 = consts.tile([128, Cm], fp32, tag="cw0")
    cw1 = consts.tile([64, Cm], fp32, tag="cw1")
    cb = consts.tile([Cm, 1], fp32, tag="cb")
    ew = consts.tile([128, 6 * E], fp32, tag="ew")
    eb = consts.tile([E, 1], fp32, tag="eb")
    ident = consts.tile([128, 128], fp32, tag="ident")
    idxt = consts.tile([128, 104], mybir.dt.int16, tag="idxt")
    zeroT = consts.tile([128, 192], bf16, tag="zeroT")

    x0 = big.tile([128, H * W], fp32, tag="x0")
    x1 = big.tile([64, H * W], fp32, tag="x1")
    t_pad2 = big.tile([128, 66 * 66], fp32, tag="tpad2")
    e_sb = big.tile([E, H * W + 4], fp32, tag="esb")
    xTe = big.tile([128, 36, C], bf16, tag="xTe")
    xTo = big.tile([128, 36, C], bf16, tag="xTo")

    # PSUM pools (8 banks total: 2+1+2+2+1)
    t5ps = es.enter_context(tc.tile_pool(name="t5ps", bufs=2, space="PSUM"))
    convps = es.enter_context(tc.tile_pool(name="convps", bufs=1, space="PSUM"))
    out0ps = es.enter_context(tc.tile_pool(name="out0ps", bufs=2, space="PSUM"))
    out1ps = es.enter_context(tc.tile_pool(name="out1ps", bufs=2, space="PSUM"))
    xtps = es.enter_context(tc.tile_pool(name="xtps", bufs=1, space="PSUM"))

    matsp = es.enter_context(tc.tile_pool(name="matsp", bufs=3))
    rsump = es.enter_context(tc.tile_pool(name="rsump", bufs=3))
    rinvp = es.enter_context(tc.tile_pool(name="rinvp", bufs=3))
    datap = es.enter_context(tc.tile_pool(name="datap", bufs=3))
    wp = es.enter_context(tc.tile_pool(name="wp", bufs=3))
    stg0p = es.enter_context(tc.tile_pool(name="stg0p", bufs=2))
    stg1p = es.enter_context(tc.tile_pool(name="stg1p", bufs=2))

    R = lambda ap: ap.bitcast(f32r)

    nc.gpsimd.load_library(library_config.local_scatter)

    # ---- input DMAs (first x chunk + conv1 weights first) ----
    nc.sync.dma_start(out=x0[:, 0:1024], in_=x_d[0:128, 0:1024])
    nc.sync.dma_start(out=x1[:, 0:1024], in_=x_d[128:192, 0:1024])
    nc.sync.dma_start(out=cw0[:], in_=cw_d[0:128, :])
    nc.sync.dma_start(out=cw1[:], in_=cw_d[128:192, :])
    nc.sync.dma_start(out=cb[:], in_=cb_d)
    for ck in range(1, 4):
        c0 = ck * 1024
        nc.sync.dma_start(out=x0[:, c0:c0 + 1024], in_=x_d[0:128, c0:c0 + 1024])
        nc.sync.dma_start(out=x1[:, c0:c0 + 1024], in_=x_d[128:192, c0:c0 + 1024])
    nc.sync.dma_start(out=ew[:], in_=ew_d)
    nc.sync.dma_start(out=eb[:], in_=eb_d)
    nc.sync.dma_start(out=ident[:], in_=id_d)
    nc.sync.dma_start(out=idxt[:], in_=idx_d)

    # ---- border memsets ----
    nc.gpsimd.memset(zeroT[:], 0.0)
    tp3 = t_pad2[:].rearrange("c (r z) -> c r z", z=66)
    nc.gpsimd.memset(tp3[0:64, 0, :], 0.0)
    nc.gpsimd.memset(tp3[0:64, 65, :], 0.0)
    nc.gpsimd.memset(tp3[:, :, 0:1], 0.0)
    nc.gpsimd.memset(tp3[:, :, 65:66], 0.0)
    nc.gpsimd.memset(e_sb[:, 0:2], 1.0)
    nc.gpsimd.memset(e_sb[:, H * W + 2:H * W + 4], 1.0)
    # zero x-row border blocks of xT (CARAFE zero padding outside the image)
    for t, b in ((xTe, 1), (xTe, 34), (xTo, 1), (xTo, 33)):
        nc.vector.tensor_copy(t[:, b, :], zeroT[:])

    # ---- helpers ----
    def conv1(nt):
        n0 = nt * 512
        ps = out1ps.tile([Cm, 512], fp32, name="c1ps", tag="o1")
        nc.tensor.matmul(ps[:], R(cw0[:]), R(x0[:, n0:n0 + 512]),
                         start=True, stop=False)
        nc.tensor.matmul(ps[:], R(cw1[:]), R(x1[:, n0:n0 + 512]),
                         start=False, stop=True)
        psv = ps[:].rearrange("c (r z) -> c r z", z=64)
        # lower half holds t rows r, upper half t rows r+1 (row-pair packing
        # for the dy in {0,1} encoder taps)
        nc.scalar.activation(out=tp3[0:64, nt * 8 + 1: nt * 8 + 9, 1:65],
                             in_=psv, func=AF.Silu, bias=cb[:], scale=1.0)
        nc.scalar.activation(out=tp3[64:128, nt * 8: nt * 8 + 8, 1:65],
                             in_=psv, func=AF.Silu, bias=cb[:], scale=1.0)

    def conv2(nt):
        r0 = nt * 8
        ps = convps.tile([E, 512], fp32, name="c2ps", tag="conv")
        for dx in range(3):
            rhs = tp3[:, r0: r0 + 8, dx: dx + 64]
            nc.tensor.matmul(ps[:], R(ew[:, dx * E:(dx + 1) * E]), R(rhs),
                             start=(dx == 0), stop=False)
        for dx in range(3):
            rhs = tp3[0:64, r0 + 2: r0 + 10, dx: dx + 64]
            nc.tensor.matmul(ps[:], R(ew[:, (3 + dx) * E:(4 + dx) * E][0:64, :]),
                             R(rhs), start=False, stop=(dx == 2))
        nc.scalar.activation(out=e_sb[:, 2 + r0 * 64: 2 + r0 * 64 + 512], in_=ps[:],
                             func=AF.Exp, bias=eb[:], scale=1.0)

    nxt = [0]  # alternate eviction engine for xT blocks

    def _xt_evict(dst, src):
        if nxt[0] % 2 == 0:
            nc.scalar.copy(out=dst, in_=src)
        else:
            nc.vector.tensor_copy(dst, src)
        nxt[0] += 1

    def xte_block(be):
        px0 = 128 * (be - 2)
        pt = xtps.tile([128, C], fp32, name="xtpt", tag="xt")
        nc.tensor.matmul(R(pt[:, 0:128]), R(x0[:, px0:px0 + 128]), R(ident[:]),
                         is_transpose=True, skip_group_check=True)
        nc.tensor.matmul(R(pt[:, 128:192]), R(x1[:, px0:px0 + 128]),
                         R(ident[0:64, 0:64]), is_transpose=True,
                         skip_group_check=True)
        _xt_evict(xTe[:, be, :], pt[:])

    def xto_block(bo):
        px0 = 128 * (bo - 2) + 64
        pt = xtps.tile([128, C], fp32, name="xopt", tag="xt")
        nc.tensor.matmul(R(pt[:, 0:128]), R(x0[:, px0:px0 + 128]), R(ident[:]),
                         is_transpose=True, skip_group_check=True)
        nc.tensor.matmul(R(pt[:, 128:192]), R(x1[:, px0:px0 + 128]),
                         R(ident[0:64, 0:64]), is_transpose=True,
                         skip_group_check=True)
        _xt_evict(xTo[:, bo, :], pt[:])

    def xto_half(bo):
        pt = xtps.tile([128, C], fp32, name="xhpt", tag="xt")
        if bo == 1:  # rows (-1, 0): only upper 64 partitions hold row 0
            cols, prt = slice(0, 64), slice(64, 128)
        else:  # bo == 33: rows (63, 64): lower 64 partitions hold row 63
            cols, prt = slice(4032, 4096), slice(0, 64)
        nc.tensor.matmul(R(pt[prt, 0:128]), R(x0[:, cols]), R(ident[:]),
                         is_transpose=True, skip_group_check=True)
        nc.tensor.matmul(R(pt[prt, 128:192]), R(x1[:, cols]),
                         R(ident[0:64, 0:64]), is_transpose=True,
                         skip_group_check=True)
        _xt_evict(xTo[prt, bo, :], pt[prt, :])

    data_tiles = {}
    w_tiles = {}

    def stageA(ti):
        # 5 shifted transposes of e_sb -> T5 [128, 5, 100] (psum), then
        # normalize into bf16 scatter data [128, 4cl, 26].
        p0 = 128 * ti
        t5 = t5ps.tile([128, 500], fp32, name="t5", tag="t5")
        for d in range(5):
            s = p0 + 4 - d
            nc.tensor.matmul(R(t5[:, d * 100:(d + 1) * 100]),
                             R(e_sb[:, s:s + 128]), R(ident[0:E, 0:E]),
                             is_transpose=True, skip_group_check=True)
        mats = matsp.tile([128, 500], bf16, name="mats", tag="mats")
        nc.scalar.copy(out=mats[:], in_=t5[:])
        rsum = rsump.tile([128, 20], fp32, name="rsum", tag="rsum")
        red_in = AP(t5[:].tensor, 0, [[500, 128], [100, 5], [1, 4], [4, 25]])
        nc.vector.tensor_reduce(out=rsum[:].rearrange("q (d c) -> q d c", c=4),
                                in_=red_in, axis=mybir.AxisListType.X, op=AL.add)
        rinv = rinvp.tile([128, 20], fp32, name="rinv", tag="rinv")
        nc.vector.reciprocal(rinv[:], rsum[:])
        data = datap.tile([128, 4, 26], bf16, name="data", tag="data")
        # data[q, cl, i*5+j] = mats[q, 104j + 20i + cl] * rinv[q, 4j + cl]
        in0 = AP(mats[:].tensor, 0, [[500, 128], [1, 4], [20, 5], [104, 5]])
        in1 = AP(rinv[:].tensor, 0, [[20, 128], [1, 4], [0, 5], [4, 5]])
        outv = AP(data[:].tensor, 0, [[104, 128], [26, 4], [5, 5], [1, 5]])
        nc.vector.tensor_tensor(outv, in0, in1, AL.mult)
        data_tiles[ti] = data

    def stageB(ti):
        # One scatter builds the whole tile's W [128, (di, i, 2w+dj)]: the
        # output-row dimension is implicit in the partition halves (q<64 ->
        # out row h0, q>=64 -> h0+1), so no structurally-zero quadrants.
        data = data_tiles.pop(ti)
        w = wp.tile([128, 2, 5, 128], bf16, name="w", tag="w")
        nc.gpsimd.local_scatter(
            out_ap=w[:].rearrange("q a b c -> q (a b c)"),
            data_ap=data[:].rearrange("q a b -> q (a b)"),
            idxs_ap=idxt[:],
            channels=128, num_elems=1280, num_idxs=104)
        w_tiles[ti] = w

    def stageC(ti):
        w = w_tiles[ti]
        for ck, (c0, cp, psp, stgp, ev) in enumerate(
                ((0, 128, out0ps, stg0p, "act"), (128, 64, out1ps, stg1p, "dve"))):
            # psum columns in output row-major order (h, di, w'): K=64
            # matmuls pair partition half hq of the x block with the same
            # half of W (out row h0+hq).
            ps = psp.tile([cp, 512], fp32, name=f"ops{ck}", tag=f"o{ck}")
            for hq, qs in ((0, slice(0, 64)), (1, slice(64, 128))):
                for di in range(2):
                    n0 = hq * 256 + di * 128
                    for i in range(5):
                        if i % 2 == 0:
                            blk = xTe[qs, ti + 1 + i // 2, c0:c0 + cp]
                        else:
                            blk = xTo[qs, ti + 1 + (i - 1) // 2, c0:c0 + cp]
                        nc.tensor.matmul(ps[:, n0:n0 + 128], blk, w[qs, di, i, :],
                                         start=(i == 0), stop=(i == 4),
                                         skip_group_check=True)
            stg = stgp.tile([cp, 512], fp32, name=f"stg{ck}", tag=f"s{ck}")
            if ev == "act":
                nc.scalar.copy(out=stg[:], in_=ps[:])
            else:
                nc.vector.tensor_copy(stg[:], ps[:])
            nc.sync.dma_start(
                out=out_d[c0:c0 + cp, 2 * ti:2 * ti + 2, :, :].rearrange(
                    "c h a w -> c (h a w)"),
                in_=stg[:])
        del w_tiles[ti]

    # ---- prologue ----
    # conv1 over the whole image (single Silu act-table load), interleaved
    # with the first xT transposes.
    for nt in range(8):
        conv1(nt)
        if nt == 0:
            xte_block(2)
        elif nt == 1:
            xte_block(3)
        elif nt == 2:
            xto_half(1)
        elif nt == 3:
            xto_block(2)
    conv2(0)
    conv2(1)
    stageA(0)
    stageA(1)
    stageA(2)
    stageB(0)
    stageB(1)

    # ---- main pipelined loop ----
    for it in range(NT):
        if it + 4 <= 33:
            xte_block(it + 4)
        if it + 3 <= 32:
            xto_block(it + 3)
        if it == 30:
            xto_half(33)
        if it % 4 == 2 and it // 4 + 2 <= 7:
            conv2(it // 4 + 2)
        if it + 3 < NT:
            stageA(it + 3)
        if it + 2 < NT:
            stageB(it + 2)
        stageC(it)
    es.pop_all().close()


def _host_prep(inputs):
    def fold(w, g, b, m, v):
        s = g / np.sqrt(v + EPS)
        return (w * s[:, None, None, None]).astype(np.float32), (b - m * s).astype(np.float32)

    comp_w_eff, comp_b_eff = fold(inputs["comp_w"], inputs["comp_g"], inputs["comp_b"],
                                  inputs["comp_m"], inputs["comp_v"])
    enc_w_eff, enc_b_eff = fold(inputs["enc_w"], inputs["enc_g"], inputs["enc_b"],
                                inputs["enc_m"], inputs["enc_v"])
    cw = np.ascontiguousarray(comp_w_eff[:, :, 0, 0].T)          # [192, 64]
    cb = comp_b_eff.reshape(Cm, 1)
    # packed encoder weights [128, 6*E]: cols dx<3 hold dy=0 (rows 0:64)
    # stacked with dy=1 (rows 64:128); cols 3+dx hold dy=2 in rows 0:64.
    ew = np.zeros((128, 6 * E), np.float32)
    for dx in range(3):
        ew[0:64, dx * E:(dx + 1) * E] = enc_w_eff[:, :, 0, dx].T
        ew[64:128, dx * E:(dx + 1) * E] = enc_w_eff[:, :, 1, dx].T
        ew[0:64, (3 + dx) * E:(4 + dx) * E] = enc_w_eff[:, :, 2, dx].T
    eb = enc_b_eff.reshape(E, 1)
    ident = np.eye(128, dtype=np.float32)
    idx = np.full((128, 104), -1, np.int16)
    for q in range(128):
        wq = q % 64
        for cl in range(4):
            di, dj = cl // 2, cl % 2
            for i in range(K):
                for j in range(K):
                    wt = wq - (j - 2)
                    if 0 <= wt < W:
                        idx[q, cl * 26 + i * 5 + j] = di * 640 + i * 128 + 2 * wt + dj
    return dict(cw=cw, cb=cb, ew=ew, eb=eb, ident=ident, idx=idx)


def kernel(**inputs):
    from concourse.bass_utils import run_bass_kernel_spmd

    inputs = {k: np.asarray(v, dtype=np.float32) if np.asarray(v).dtype != np.int16
              else np.asarray(v) for k, v in inputs.items()}
    w = _host_prep(inputs)
    if "nc" not in _prog_cache:
        _prog_cache["nc"] = _build_program()
    nc = _prog_cache["nc"]
    x = inputs["x"]
    in_maps = [dict(x=np.ascontiguousarray(x[b].reshape(C, H * W)), **w) for b in range(B)]
    res = run_bass_kernel_spmd(nc, in_maps, list(range(B)))
    out = np.stack([res.results[b]["out"].reshape(C, 2 * H, 2 * W) for b in range(B)])
    return out


# revision 11
# speedup vs baseline: 6.2266x; 1.0286x over previous
# CARAFE (content-aware reassembly) Trainium2 Bass kernel, v2.
# Strategy: data-parallel over batch (8 items -> 8 NeuronCores). Per core:
#   - 1x1 compressor conv + folded BN + SiLU entirely via PE fp32r matmuls
#     (N=512 -> 1 cyc/row) + one ACT Silu per tile.
#   - 3x3 encoder conv as 9 accumulating fp32r matmuls + ACT Exp -> e_sb
#     [100ch, pix] channel-major exp(mask) tensor.
#   - Reassembly on PE in bf16: for each 2-row pixel tile and subpixel-class
#     pair, out[c, p] = sum_i xT_block_i^T @ W_i where W_i are [128, 256]
#     banded matrices holding softmax-normalized mask values on diagonals
#     j-2 in {-2..2}. W is built by ONE gpsimd local_scatter per (tile,
#     class-pair) from data produced with zero partition-shifts:
#     PE-transposing column-shifted slices of e_sb yields all shifted mask
#     columns; constant int16 scatter indices encode tap geometry and edge
#     clipping (idx=-1 drops out-of-image taps, matching x zero-padding).
#   - Softmax normalization: DVE reduce over the transposed (shifted) mask
#     + reciprocal, folded into the scatter data via one strided
#     tensor_tensor (custom APs).
#   - Output accumulated in PSUM [c, (di, h, 2w+dj)], evicted (ACT/DVE) and
#     DMA'd in channel-major order.
import sys
import numpy as np

for _p in ("/opt/trn_rl_repo",):
    if _p not in sys.path:
        sys.path.insert(0, _p)

B, C, Cm, E = 8, 192, 64, 100
H = W = 64
K, S = 5, 2
EPS = 1e-3
NT = 32  # 2-row pixel tiles

_prog_cache = {}


def _build_program(num_devices=8):
    import concourse.mybir as mybir
    import concourse.tile as tile
    from concourse import bacc
    from contextlib import ExitStack

    fp32 = mybir.dt.float32
    nc = bacc.Bacc("TRN2", target_bir_lowering=False, num_devices=num_devices)

    x_d = nc.dram_tensor("x", [C, H * W], fp32, kind="ExternalInput").ap()
    cw_d = nc.dram_tensor("cw", [C, Cm], fp32, kind="ExternalInput").ap()
    cb_d = nc.dram_tensor("cb", [Cm, 1], fp32, kind="ExternalInput").ap()
    ew_d = nc.dram_tensor("ew", [128, 6 * E], fp32, kind="ExternalInput").ap()
    eb_d = nc.dram_tensor("eb", [E, 1], fp32, kind="ExternalInput").ap()
    id_d = nc.dram_tensor("ident", [128, 128], fp32, kind="ExternalInput").ap()
    idb_d = nc.dram_tensor("identb", [128, 128], mybir.dt.bfloat16, kind="ExternalInput").ap()
    idx_d = nc.dram_tensor("idx", [128, 104], mybir.dt.int16, kind="ExternalInput").ap()
    out_d = nc.dram_tensor("out", [C, H, 2, 2 * W], fp32, kind="ExternalOutput").ap()

    es = ExitStack()
    with tile.TileContext(nc) as tc:
        with es:
            _body(es, tc, nc, mybir,
                  x_d, cw_d, cb_d, ew_d, eb_d, id_d, idb_d, idx_d, out_d)
    nc.compile()
    return nc


def _body(es, tc, nc, mybir, x_d, cw_d, cb_d, ew_d, eb_d, id_d, idb_d, idx_d, out_d):
    from concourse.ap import AP
    from concourse import library_config

    fp32 = mybir.dt.float32
    f32r = mybir.dt.float32r
    bf16 = mybir.dt.bfloat16
    AL = mybir.AluOpType
    AF = mybir.ActivationFunctionType

    consts = es.enter_context(tc.tile_pool(name="consts", bufs=1))
    big = es.enter_context(tc.tile_pool(name="big", bufs=1))

    cw0 = consts.tile([128, Cm], fp32, tag="cw0")
    cw1 = consts.tile([64, Cm], fp32, tag="cw1")
    cb = consts.tile([Cm, 1], fp32, tag="cb")
    ew = consts.tile([128, 6 * E], fp32, tag="ew")
    eb = consts.tile([E, 1], fp32, tag="eb")
    ident = consts.tile([128, 128], fp32, tag="ident")
    identb = consts.tile([128, 128], bf16, tag="identb")
    idxt = consts.tile([128, 104], mybir.dt.int16, tag="idxt")
    zeroT = consts.tile([128, 192], bf16, tag="zeroT")

    x0 = big.tile([128, H * W], fp32, tag="x0")
    x1 = big.tile([64, H * W], fp32, tag="x1")
    t_pad2 = big.tile([128, 66 * 66], fp32, tag="tpad2")
    e_sb = big.tile([E, H * W + 4], bf16, tag="esb")
    xTe = big.tile([128, 36, C], bf16, tag="xTe")
    xTo = big.tile([128, 36, C], bf16, tag="xTo")

    # PSUM pools (8 banks total: 2+1+2+2+1)
    t5ps = es.enter_context(tc.tile_pool(name="t5ps", bufs=2, space="PSUM"))
    convps = es.enter_context(tc.tile_pool(name="convps", bufs=1, space="PSUM"))
    out0ps = es.enter_context(tc.tile_pool(name="out0ps", bufs=2, space="PSUM"))
    out1ps = es.enter_context(tc.tile_pool(name="out1ps", bufs=2, space="PSUM"))
    xtps = es.enter_context(tc.tile_pool(name="xtps", bufs=1, space="PSUM"))

    matsp = es.enter_context(tc.tile_pool(name="matsp", bufs=3))
    rsump = es.enter_context(tc.tile_pool(name="rsump", bufs=3))
    rinvp = es.enter_context(tc.tile_pool(name="rinvp", bufs=3))
    datap = es.enter_context(tc.tile_pool(name="datap", bufs=3))
    wp = es.enter_context(tc.tile_pool(name="wp", bufs=3))
    stg0p = es.enter_context(tc.tile_pool(name="stg0p", bufs=2))
    stg1p = es.enter_context(tc.tile_pool(name="stg1p", bufs=2))

    R = lambda ap: ap.bitcast(f32r)

    nc.gpsimd.load_library(library_config.local_scatter)

    # ---- input DMAs (first x chunk + conv1 weights first) ----
    nc.sync.dma_start(out=x0[:, 0:1024], in_=x_d[0:128, 0:1024])
    nc.sync.dma_start(out=x1[:, 0:1024], in_=x_d[128:192, 0:1024])
    nc.sync.dma_start(out=cw0[:], in_=cw_d[0:128, :])
    nc.sync.dma_start(out=cw1[:], in_=cw_d[128:192, :])
    nc.sync.dma_start(out=cb[:], in_=cb_d)
    for ck in range(1, 4):
        c0 = ck * 1024
        nc.sync.dma_start(out=x0[:, c0:c0 + 1024], in_=x_d[0:128, c0:c0 + 1024])
        nc.sync.dma_start(out=x1[:, c0:c0 + 1024], in_=x_d[128:192, c0:c0 + 1024])
    nc.sync.dma_start(out=ew[:], in_=ew_d)
    nc.sync.dma_start(out=eb[:], in_=eb_d)
    nc.sync.dma_start(out=ident[:], in_=id_d)
    nc.sync.dma_start(out=identb[:], in_=idb_d)
    nc.sync.dma_start(out=idxt[:], in_=idx_d)

    # ---- border memsets ----
    nc.gpsimd.memset(zeroT[:], 0.0)
    tp3 = t_pad2[:].rearrange("c (r z) -> c r z", z=66)
    nc.gpsimd.memset(tp3[0:64, 0, :], 0.0)
    nc.gpsimd.memset(tp3[0:64, 65, :], 0.0)
    nc.gpsimd.memset(tp3[:, :, 0:1], 0.0)
    nc.gpsimd.memset(tp3[:, :, 65:66], 0.0)
    nc.gpsimd.memset(e_sb[:, 0:2], 1.0)
    nc.gpsimd.memset(e_sb[:, H * W + 2:H * W + 4], 1.0)
    # zero x-row border blocks of xT (CARAFE zero padding outside the image)
    for t, b in ((xTe, 1), (xTe, 34), (xTo, 1), (xTo, 33)):
        nc.vector.tensor_copy(t[:, b, :], zeroT[:])

    # ---- helpers ----
    def conv1(nt):
        n0 = nt * 512
        ps = out1ps.tile([Cm, 512], fp32, name="c1ps", tag="o1")
        nc.tensor.matmul(ps[:], R(cw0[:]), R(x0[:, n0:n0 + 512]),
                         start=True, stop=False)
        nc.tensor.matmul(ps[:], R(cw1[:]), R(x1[:, n0:n0 + 512]),
                         start=False, stop=True)
        psv = ps[:].rearrange("c (r z) -> c r z", z=64)
        # lower half holds t rows r, upper half t rows r+1 (row-pair packing
        # for the dy in {0,1} encoder taps)
        nc.scalar.activation(out=tp3[0:64, nt * 8 + 1: nt * 8 + 9, 1:65],
                             in_=psv, func=AF.Silu, bias=cb[:], scale=1.0)
        nc.scalar.activation(out=tp3[64:128, nt * 8: nt * 8 + 8, 1:65],
                             in_=psv, func=AF.Silu, bias=cb[:], scale=1.0)

    def conv2(nt):
        r0 = nt * 8
        ps = convps.tile([E, 512], fp32, name="c2ps", tag="conv")
        for dx in range(3):
            rhs = tp3[:, r0: r0 + 8, dx: dx + 64]
            nc.tensor.matmul(ps[:], R(ew[:, dx * E:(dx + 1) * E]), R(rhs),
                             start=(dx == 0), stop=False)
        for dx in range(3):
            rhs = tp3[0:64, r0 + 2: r0 + 10, dx: dx + 64]
            nc.tensor.matmul(ps[:], R(ew[:, (3 + dx) * E:(4 + dx) * E][0:64, :]),
                             R(rhs), start=False, stop=(dx == 2))
        nc.scalar.activation(out=e_sb[:, 2 + r0 * 64: 2 + r0 * 64 + 512], in_=ps[:],
                             func=AF.Exp, bias=eb[:], scale=1.0)

    nxt = [0]  # alternate eviction engine for xT blocks

    def _xt_evict(dst, src):
        if nxt[0] % 2 == 0:
            nc.scalar.copy(out=dst, in_=src)
        else:
            nc.vector.tensor_copy(dst, src)
        nxt[0] += 1

    def xte_block(be):
        px0 = 128 * (be - 2)
        pt = xtps.tile([128, C], fp32, name="xtpt", tag="xt")
        nc.tensor.matmul(R(pt[:, 0:128]), R(x0[:, px0:px0 + 128]), R(ident[:]),
                         is_transpose=True, skip_group_check=True)
        nc.tensor.matmul(R(pt[:, 128:192]), R(x1[:, px0:px0 + 128]),
                         R(ident[0:64, 0:64]), is_transpose=True,
                         skip_group_check=True)
        _xt_evict(xTe[:, be, :], pt[:])

    def xto_block(bo):
        px0 = 128 * (bo - 2) + 64
        pt = xtps.tile([128, C], fp32, name="xopt", tag="xt")
        nc.tensor.matmul(R(pt[:, 0:128]), R(x0[:, px0:px0 + 128]), R(ident[:]),
                         is_transpose=True, skip_group_check=True)
        nc.tensor.matmul(R(pt[:, 128:192]), R(x1[:, px0:px0 + 128]),
                         R(ident[0:64, 0:64]), is_transpose=True,
                         skip_group_check=True)
        _xt_evict(xTo[:, bo, :], pt[:])

    def xto_half(bo):
        pt = xtps.tile([128, C], fp32, name="xhpt", tag="xt")
        if bo == 1:  # rows (-1, 0): only upper 64 partitions hold row 0
            cols, prt = slice(0, 64), slice(64, 128)
        else:  # bo == 33: rows (63, 64): lower 64 partitions hold row 63
            cols, prt = slice(4032, 4096), slice(0, 64)
        nc.tensor.matmul(R(pt[prt, 0:128]), R(x0[:, cols]), R(ident[:]),
                         is_transpose=True, skip_group_check=True)
        nc.tensor.matmul(R(pt[prt, 128:192]), R(x1[:, cols]),
                         R(ident[0:64, 0:64]), is_transpose=True,
                         skip_group_check=True)
        _xt_evict(xTo[prt, bo, :], pt[prt, :])

    data_tiles = {}
    w_tiles = {}

    def stageA(ti):
        # 5 shifted transposes of e_sb -> T5 [128, 5, 100] (psum), then
        # normalize into bf16 scatter data [128, 4cl, 26].
        p0 = 128 * ti
        t5 = t5ps.tile([128, 500], bf16, name="t5", tag="t5")
        for d in range(5):
            s = p0 + 4 - d
            nc.tensor.matmul(t5[:, d * 100:(d + 1) * 100],
                             e_sb[:, s:s + 128], identb[0:E, 0:E],
                             is_transpose=True, skip_group_check=True)
        mats = matsp.tile([128, 500], bf16, name="mats", tag="mats")
        nc.scalar.copy(out=mats[:], in_=t5[:])
        rsum = rsump.tile([128, 20], fp32, name="rsum", tag="rsum")
        red_in = AP(t5[:].tensor, 0, [[500, 128], [100, 5], [1, 4], [4, 25]])
        nc.vector.tensor_reduce(out=rsum[:].rearrange("q (d c) -> q d c", c=4),
                                in_=red_in, axis=mybir.AxisListType.X, op=AL.add)
        rinv = rinvp.tile([128, 20], fp32, name="rinv", tag="rinv")
        nc.vector.reciprocal(rinv[:], rsum[:])
        data = datap.tile([128, 4, 26], bf16, name="data", tag="data")
        # data[q, cl, i*5+j] = mats[q, 104j + 20i + cl] * rinv[q, 4j + cl]
        in0 = AP(mats[:].tensor, 0, [[500, 128], [1, 4], [20, 5], [104, 5]])
        in1 = AP(rinv[:].tensor, 0, [[20, 128], [1, 4], [0, 5], [4, 5]])
        outv = AP(data[:].tensor, 0, [[104, 128], [26, 4], [5, 5], [1, 5]])
        nc.vector.tensor_tensor(outv, in0, in1, AL.mult)
        data_tiles[ti] = data

    def stageB(ti):
        # One scatter builds the whole tile's W [128, (di, i, 2w+dj)]: the
        # output-row dimension is implicit in the partition halves (q<64 ->
        # out row h0, q>=64 -> h0+1), so no structurally-zero quadrants.
        data = data_tiles.pop(ti)
        w = wp.tile([128, 2, 5, 128], bf16, name="w", tag="w")
        nc.gpsimd.local_scatter(
            out_ap=w[:].rearrange("q a b c -> q (a b c)"),
            data_ap=data[:].rearrange("q a b -> q (a b)"),
            idxs_ap=idxt[:],
            channels=128, num_elems=1280, num_idxs=104)
        w_tiles[ti] = w

    def stageC(ti):
        w = w_tiles[ti]
        for ck, (c0, cp, psp, stgp, ev) in enumerate(
                ((0, 128, out0ps, stg0p, "act"), (128, 64, out1ps, stg1p, "dve"))):
            # psum columns in output row-major order (h, di, w'): K=64
            # matmuls pair partition half hq of the x block with the same
            # half of W (out row h0+hq).
            ps = psp.tile([cp, 512], fp32, name=f"ops{ck}", tag=f"o{ck}")
            for hq, qs in ((0, slice(0, 64)), (1, slice(64, 128))):
                for di in range(2):
                    n0 = hq * 256 + di * 128
                    for i in range(5):
                        if i % 2 == 0:
                            blk = xTe[qs, ti + 1 + i // 2, c0:c0 + cp]
                        else:
                            blk = xTo[qs, ti + 1 + (i - 1) // 2, c0:c0 + cp]
                        nc.tensor.matmul(ps[:, n0:n0 + 128], blk, w[qs, di, i, :],
                                         start=(i == 0), stop=(i == 4),
                                         skip_group_check=True)
            stg = stgp.tile([cp, 512], fp32, name=f"stg{ck}", tag=f"s{ck}")
            if ev == "act":
                nc.scalar.copy(out=stg[:], in_=ps[:])
            else:
                nc.vector.tensor_copy(stg[:], ps[:])
            nc.sync.dma_start(
                out=out_d[c0:c0 + cp, 2 * ti:2 * ti + 2, :, :].rearrange(
                    "c h a w -> c (h a w)"),
                in_=stg[:])
        del w_tiles[ti]

    # ---- prologue ----
    # conv1 over the whole image (single Silu act-table load), interleaved
    # with the first xT transposes.
    for nt in range(8):
        conv1(nt)
        xte_block(nt + 2) if nt < 4 else xto_block(nt - 2)
        if nt == 7:
            xto_half(1)
    conv2(0)
    conv2(1)
    stageA(0)
    stageA(1)
    stageA(2)
    stageB(0)
    stageB(1)

    # ---- main pipelined loop ----
    for it in range(NT):
        if it % 4 == 2 and it // 4 + 2 <= 7:
            conv2(it // 4 + 2)
        if it + 3 < NT:
            stageA(it + 3)
        if it + 2 < NT:
            stageB(it + 2)
        stageC(it)
        if it + 6 <= 33:
            xte_block(it + 6)
        if it + 6 <= 32:
            xto_block(it + 6)
        if it == 27:
            xto_half(33)
    es.pop_all().close()


def _host_prep(inputs):
    def fold(w, g, b, m, v):
        s = g / np.sqrt(v + EPS)
        return (w * s[:, None, None, None]).astype(np.float32), (b - m * s).astype(np.float32)

    comp_w_eff, comp_b_eff = fold(inputs["comp_w"], inputs["comp_g"], inputs["comp_b"],
                                  inputs["comp_m"], inputs["comp_v"])
    enc_w_eff, enc_b_eff = fold(inputs["enc_w"], inputs["enc_g"], inputs["enc_b"],
                                inputs["enc_m"], inputs["enc_v"])
    cw = np.ascontiguousarray(comp_w_eff[:, :, 0, 0].T)          # [192, 64]
    cb = comp_b_eff.reshape(Cm, 1)
    # packed encoder weights [128, 6*E]: cols dx<3 hold dy=0 (rows 0:64)
    # stacked with dy=1 (rows 64:128); cols 3+dx hold dy=2 in rows 0:64.
    ew = np.zeros((128, 6 * E), np.float32)
    for dx in range(3):
        ew[0:64, dx * E:(dx + 1) * E] = enc_w_eff[:, :, 0, dx].T
        ew[64:128, dx * E:(dx + 1) * E] = enc_w_eff[:, :, 1, dx].T
        ew[0:64, (3 + dx) * E:(4 + dx) * E] = enc_w_eff[:, :, 2, dx].T
    eb = enc_b_eff.reshape(E, 1)
    ident = np.eye(128, dtype=np.float32)
    idx = np.full((128, 104), -1, np.int16)
    for q in range(128):
        wq = q % 64
        for cl in range(4):
            di, dj = cl // 2, cl % 2
            for i in range(K):
                for j in range(K):
                    wt = wq - (j - 2)
                    if 0 <= wt < W:
                        idx[q, cl * 26 + i * 5 + j] = di * 640 + i * 128 + 2 * wt + dj
    return dict(cw=cw, cb=cb, ew=ew, eb=eb, ident=ident, idx=idx)


def kernel(**inputs):
    from concourse.bass_utils import run_bass_kernel_spmd

    inputs = {k: np.asarray(v, dtype=np.float32) if np.asarray(v).dtype != np.int16
              else np.asarray(v) for k, v in inputs.items()}
    w = _host_prep(inputs)
    if "nc" not in _prog_cache:
        _prog_cache["nc"] = _build_program()
    nc = _prog_cache["nc"]
    x = inputs["x"]
    in_maps = [dict(x=np.ascontiguousarray(x[b].reshape(C, H * W)), **w) for b in range(B)]
    res = run_bass_kernel_spmd(nc, in_maps, list(range(B)))
    out = np.stack([res.results[b]["out"].reshape(C, 2 * H, 2 * W) for b in range(B)])
    return out


# revision 13
# speedup vs baseline: 6.3312x; 1.0168x over previous
# CARAFE (content-aware reassembly) Trainium2 Bass kernel, v2.
# Strategy: data-parallel over batch (8 items -> 8 NeuronCores). Per core:
#   - 1x1 compressor conv + folded BN + SiLU entirely via PE fp32r matmuls
#     (N=512 -> 1 cyc/row) + one ACT Silu per tile.
#   - 3x3 encoder conv as 9 accumulating fp32r matmuls + ACT Exp -> e_sb
#     [100ch, pix] channel-major exp(mask) tensor.
#   - Reassembly on PE in bf16: for each 2-row pixel tile and subpixel-class
#     pair, out[c, p] = sum_i xT_block_i^T @ W_i where W_i are [128, 256]
#     banded matrices holding softmax-normalized mask values on diagonals
#     j-2 in {-2..2}. W is built by ONE gpsimd local_scatter per (tile,
#     class-pair) from data produced with zero partition-shifts:
#     PE-transposing column-shifted slices of e_sb yields all shifted mask
#     columns; constant int16 scatter indices encode tap geometry and edge
#     clipping (idx=-1 drops out-of-image taps, matching x zero-padding).
#   - Softmax normalization: DVE reduce over the transposed (shifted) mask
#     + reciprocal, folded into the scatter data via one strided
#     tensor_tensor (custom APs).
#   - Output accumulated in PSUM [c, (di, h, 2w+dj)], evicted (ACT/DVE) and
#     DMA'd in channel-major order.
import sys
import numpy as np

for _p in ("/opt/trn_rl_repo",):
    if _p not in sys.path:
        sys.path.insert(0, _p)

B, C, Cm, E = 8, 192, 64, 100
H = W = 64
K, S = 5, 2
EPS = 1e-3
NT = 32  # 2-row pixel tiles

_prog_cache = {}


def _build_program(num_devices=8):
    import concourse.mybir as mybir
    import concourse.tile as tile
    from concourse import bacc
    from contextlib import ExitStack

    fp32 = mybir.dt.float32
    nc = bacc.Bacc("TRN2", target_bir_lowering=False, num_devices=num_devices)

    x_d = nc.dram_tensor("x", [C, H * W], fp32, kind="ExternalInput").ap()
    cw_d = nc.dram_tensor("cw", [C, Cm], fp32, kind="ExternalInput").ap()
    cb_d = nc.dram_tensor("cb", [Cm, 1], fp32, kind="ExternalInput").ap()
    ew_d = nc.dram_tensor("ew", [128, 6 * E], fp32, kind="ExternalInput").ap()
    eb_d = nc.dram_tensor("eb", [E, 1], fp32, kind="ExternalInput").ap()
    id_d = nc.dram_tensor("ident", [128, 128], fp32, kind="ExternalInput").ap()
    idb_d = nc.dram_tensor("identb", [128, 128], mybir.dt.bfloat16, kind="ExternalInput").ap()
    idx_d = nc.dram_tensor("idx", [128, 104], mybir.dt.int16, kind="ExternalInput").ap()
    out_d = nc.dram_tensor("out", [C, H, 2, 2 * W], fp32, kind="ExternalOutput").ap()

    es = ExitStack()
    with tile.TileContext(nc) as tc:
        with es:
            _body(es, tc, nc, mybir,
                  x_d, cw_d, cb_d, ew_d, eb_d, id_d, idb_d, idx_d, out_d)
    nc.compile()
    return nc


def _body(es, tc, nc, mybir, x_d, cw_d, cb_d, ew_d, eb_d, id_d, idb_d, idx_d, out_d):
    from concourse.ap import AP
    from concourse import library_config

    fp32 = mybir.dt.float32
    f32r = mybir.dt.float32r
    bf16 = mybir.dt.bfloat16
    AL = mybir.AluOpType
    AF = mybir.ActivationFunctionType

    consts = es.enter_context(tc.tile_pool(name="consts", bufs=1))
    big = es.enter_context(tc.tile_pool(name="big", bufs=1))

    cw0 = consts.tile([128, Cm], fp32, tag="cw0")
    cw1 = consts.tile([64, Cm], fp32, tag="cw1")
    cb = consts.tile([Cm, 1], fp32, tag="cb")
    ew = consts.tile([128, 6 * E], fp32, tag="ew")
    eb = consts.tile([E, 1], fp32, tag="eb")
    ident = consts.tile([128, 128], fp32, tag="ident")
    identb = consts.tile([128, 128], bf16, tag="identb")
    idxt = consts.tile([128, 104], mybir.dt.int16, tag="idxt")
    zeroT = consts.tile([128, 192], bf16, tag="zeroT")

    x0 = big.tile([128, H * W], fp32, tag="x0")
    x1 = big.tile([64, H * W], fp32, tag="x1")
    t_pad2 = big.tile([128, 66 * 66], fp32, tag="tpad2")
    e_sb = big.tile([E, H * W + 4], bf16, tag="esb")
    xTe = big.tile([128, 36, C], bf16, tag="xTe")
    xTo = big.tile([128, 36, C], bf16, tag="xTo")

    # PSUM pools (8 banks total: 2+1+2+2+1)
    t5ps = es.enter_context(tc.tile_pool(name="t5ps", bufs=2, space="PSUM"))
    convps = es.enter_context(tc.tile_pool(name="convps", bufs=1, space="PSUM"))
    out0ps = es.enter_context(tc.tile_pool(name="out0ps", bufs=2, space="PSUM"))
    out1ps = es.enter_context(tc.tile_pool(name="out1ps", bufs=2, space="PSUM"))
    xtps = es.enter_context(tc.tile_pool(name="xtps", bufs=1, space="PSUM"))

    matsp = es.enter_context(tc.tile_pool(name="matsp", bufs=3))
    rsump = es.enter_context(tc.tile_pool(name="rsump", bufs=3))
    rinvp = es.enter_context(tc.tile_pool(name="rinvp", bufs=3))
    datap = es.enter_context(tc.tile_pool(name="datap", bufs=3))
    wp = es.enter_context(tc.tile_pool(name="wp", bufs=4))
    stg0p = es.enter_context(tc.tile_pool(name="stg0p", bufs=2))
    stg1p = es.enter_context(tc.tile_pool(name="stg1p", bufs=2))

    R = lambda ap: ap.bitcast(f32r)

    nc.gpsimd.load_library(library_config.local_scatter)

    # ---- input DMAs (first x chunk + conv1 weights first) ----
    nc.sync.dma_start(out=x0[:, 0:1024], in_=x_d[0:128, 0:1024])
    nc.sync.dma_start(out=x1[:, 0:1024], in_=x_d[128:192, 0:1024])
    nc.sync.dma_start(out=cw0[:], in_=cw_d[0:128, :])
    nc.sync.dma_start(out=cw1[:], in_=cw_d[128:192, :])
    nc.sync.dma_start(out=cb[:], in_=cb_d)
    for ck in range(1, 4):
        c0 = ck * 1024
        nc.sync.dma_start(out=x0[:, c0:c0 + 1024], in_=x_d[0:128, c0:c0 + 1024])
        nc.sync.dma_start(out=x1[:, c0:c0 + 1024], in_=x_d[128:192, c0:c0 + 1024])
    nc.sync.dma_start(out=ew[:], in_=ew_d)
    nc.sync.dma_start(out=eb[:], in_=eb_d)
    nc.sync.dma_start(out=ident[:], in_=id_d)
    nc.sync.dma_start(out=identb[:], in_=idb_d)
    nc.sync.dma_start(out=idxt[:], in_=idx_d)

    # ---- border memsets ----
    nc.gpsimd.memset(zeroT[:], 0.0)
    tp3 = t_pad2[:].rearrange("c (r z) -> c r z", z=66)
    nc.gpsimd.memset(tp3[0:64, 0, :], 0.0)
    nc.gpsimd.memset(tp3[0:64, 65, :], 0.0)
    nc.gpsimd.memset(tp3[:, :, 0:1], 0.0)
    nc.gpsimd.memset(tp3[:, :, 65:66], 0.0)
    nc.gpsimd.memset(e_sb[:, 0:2], 1.0)
    nc.gpsimd.memset(e_sb[:, H * W + 2:H * W + 4], 1.0)
    # zero x-row border blocks of xT (CARAFE zero padding outside the image)
    for t, b in ((xTe, 1), (xTe, 34), (xTo, 1), (xTo, 33)):
        nc.vector.tensor_copy(t[:, b, :], zeroT[:])

    # ---- helpers ----
    def conv1(nt):
        n0 = nt * 512
        ps = out1ps.tile([Cm, 512], fp32, name="c1ps", tag="o1")
        nc.tensor.matmul(ps[:], R(cw0[:]), R(x0[:, n0:n0 + 512]),
                         start=True, stop=False)
        nc.tensor.matmul(ps[:], R(cw1[:]), R(x1[:, n0:n0 + 512]),
                         start=False, stop=True)
        psv = ps[:].rearrange("c (r z) -> c r z", z=64)
        # lower half holds t rows r, upper half t rows r+1 (row-pair packing
        # for the dy in {0,1} encoder taps)
        nc.scalar.activation(out=tp3[0:64, nt * 8 + 1: nt * 8 + 9, 1:65],
                             in_=psv, func=AF.Silu, bias=cb[:], scale=1.0)
        nc.scalar.activation(out=tp3[64:128, nt * 8: nt * 8 + 8, 1:65],
                             in_=psv, func=AF.Silu, bias=cb[:], scale=1.0)

    def conv2(nt):
        r0 = nt * 8
        ps = convps.tile([E, 512], fp32, name="c2ps", tag="conv")
        for dx in range(3):
            rhs = tp3[:, r0: r0 + 8, dx: dx + 64]
            nc.tensor.matmul(ps[:], R(ew[:, dx * E:(dx + 1) * E]), R(rhs),
                             start=(dx == 0), stop=False)
        for dx in range(3):
            rhs = tp3[0:64, r0 + 2: r0 + 10, dx: dx + 64]
            nc.tensor.matmul(ps[:], R(ew[:, (3 + dx) * E:(4 + dx) * E][0:64, :]),
                             R(rhs), start=False, stop=(dx == 2))
        nc.scalar.activation(out=e_sb[:, 2 + r0 * 64: 2 + r0 * 64 + 512], in_=ps[:],
                             func=AF.Exp, bias=eb[:], scale=1.0)

    nxt = [0]  # alternate eviction engine for xT blocks

    def _xt_evict(dst, src):
        if nxt[0] % 2 == 0:
            nc.scalar.copy(out=dst, in_=src)
        else:
            nc.vector.tensor_copy(dst, src)
        nxt[0] += 1

    def xte_block(be):
        px0 = 128 * (be - 2)
        pt = xtps.tile([128, C], fp32, name="xtpt", tag="xt")
        nc.tensor.matmul(R(pt[:, 0:128]), R(x0[:, px0:px0 + 128]), R(ident[:]),
                         is_transpose=True, skip_group_check=True)
        nc.tensor.matmul(R(pt[:, 128:192]), R(x1[:, px0:px0 + 128]),
                         R(ident[0:64, 0:64]), is_transpose=True,
                         skip_group_check=True)
        _xt_evict(xTe[:, be, :], pt[:])

    def xto_block(bo):
        px0 = 128 * (bo - 2) + 64
        pt = xtps.tile([128, C], fp32, name="xopt", tag="xt")
        nc.tensor.matmul(R(pt[:, 0:128]), R(x0[:, px0:px0 + 128]), R(ident[:]),
                         is_transpose=True, skip_group_check=True)
        nc.tensor.matmul(R(pt[:, 128:192]), R(x1[:, px0:px0 + 128]),
                         R(ident[0:64, 0:64]), is_transpose=True,
                         skip_group_check=True)
        _xt_evict(xTo[:, bo, :], pt[:])

    def xto_half(bo):
        pt = xtps.tile([128, C], fp32, name="xhpt", tag="xt")
        if bo == 1:  # rows (-1, 0): only upper 64 partitions hold row 0
            cols, prt = slice(0, 64), slice(64, 128)
        else:  # bo == 33: rows (63, 64): lower 64 partitions hold row 63
            cols, prt = slice(4032, 4096), slice(0, 64)
        nc.tensor.matmul(R(pt[prt, 0:128]), R(x0[:, cols]), R(ident[:]),
                         is_transpose=True, skip_group_check=True)
        nc.tensor.matmul(R(pt[prt, 128:192]), R(x1[:, cols]),
                         R(ident[0:64, 0:64]), is_transpose=True,
                         skip_group_check=True)
        _xt_evict(xTo[prt, bo, :], pt[prt, :])

    data_tiles = {}
    w_tiles = {}

    def stageA(ti):
        # 5 shifted transposes of e_sb -> T5 [128, 5, 100] (psum), then
        # normalize into bf16 scatter data [128, 4cl, 26].
        p0 = 128 * ti
        t5 = t5ps.tile([128, 500], bf16, name="t5", tag="t5")
        for d in range(5):
            s = p0 + 4 - d
            nc.tensor.matmul(t5[:, d * 100:(d + 1) * 100],
                             e_sb[:, s:s + 128], identb[0:E, 0:E],
                             is_transpose=True, skip_group_check=True)
        mats = matsp.tile([128, 500], bf16, name="mats", tag="mats")
        nc.scalar.copy(out=mats[:], in_=t5[:])
        rsum = rsump.tile([128, 20], fp32, name="rsum", tag="rsum")
        red_in = AP(t5[:].tensor, 0, [[500, 128], [100, 5], [1, 4], [4, 25]])
        nc.vector.tensor_reduce(out=rsum[:].rearrange("q (d c) -> q d c", c=4),
                                in_=red_in, axis=mybir.AxisListType.X, op=AL.add)
        rinv = rinvp.tile([128, 20], fp32, name="rinv", tag="rinv")
        nc.vector.reciprocal(rinv[:], rsum[:])
        data = datap.tile([128, 4, 26], bf16, name="data", tag="data")
        # data[q, cl, i*5+j] = mats[q, 104j + 20i + cl] * rinv[q, 4j + cl]
        in0 = AP(mats[:].tensor, 0, [[500, 128], [1, 4], [20, 5], [104, 5]])
        in1 = AP(rinv[:].tensor, 0, [[20, 128], [1, 4], [0, 5], [4, 5]])
        outv = AP(data[:].tensor, 0, [[104, 128], [26, 4], [5, 5], [1, 5]])
        nc.vector.tensor_tensor(outv, in0, in1, AL.mult)
        data_tiles[ti] = data

    def stageB(ti):
        # One scatter builds the whole tile's W [128, (di, i, 2w+dj)]: the
        # output-row dimension is implicit in the partition halves (q<64 ->
        # out row h0, q>=64 -> h0+1), so no structurally-zero quadrants.
        data = data_tiles.pop(ti)
        w = wp.tile([128, 2, 5, 128], bf16, name="w", tag="w")
        nc.gpsimd.local_scatter(
            out_ap=w[:].rearrange("q a b c -> q (a b c)"),
            data_ap=data[:].rearrange("q a b -> q (a b)"),
            idxs_ap=idxt[:],
            channels=128, num_elems=1280, num_idxs=104)
        w_tiles[ti] = w

    def stageC(ti):
        w = w_tiles[ti]
        for ck, (c0, cp, psp, stgp, ev) in enumerate(
                ((0, 128, out0ps, stg0p, "act"), (128, 64, out1ps, stg1p, "dve"))):
            # psum columns in output row-major order (h, di, w'): K=64
            # matmuls pair partition half hq of the x block with the same
            # half of W (out row h0+hq).
            ps = psp.tile([cp, 512], fp32, name=f"ops{ck}", tag=f"o{ck}")
            for hq, qs in ((0, slice(0, 64)), (1, slice(64, 128))):
                for di in range(2):
                    n0 = hq * 256 + di * 128
                    for i in range(5):
                        if i % 2 == 0:
                            blk = xTe[qs, ti + 1 + i // 2, c0:c0 + cp]
                        else:
                            blk = xTo[qs, ti + 1 + (i - 1) // 2, c0:c0 + cp]
                        nc.tensor.matmul(ps[:, n0:n0 + 128], blk, w[qs, di, i, :],
                                         start=(i == 0), stop=(i == 4),
                                         skip_group_check=True)
            stg = stgp.tile([cp, 512], fp32, name=f"stg{ck}", tag=f"s{ck}")
            if ev == "act":
                nc.scalar.copy(out=stg[:], in_=ps[:])
            else:
                nc.vector.tensor_copy(stg[:], ps[:])
            nc.sync.dma_start(
                out=out_d[c0:c0 + cp, 2 * ti:2 * ti + 2, :, :].rearrange(
                    "c h a w -> c (h a w)"),
                in_=stg[:])
        del w_tiles[ti]

    # ---- prologue ----
    # start the reassembly pipeline as soon as conv coverage allows; conv1
    # runs ahead of conv2 by one tile (all Silu first would idle the PE on
    # input DMAs).
    conv1(0)
    conv1(1)
    xte_block(2)
    xte_block(3)
    xto_half(1)
    xto_block(2)
    conv2(0)
    conv1(2)
    conv2(1)
    xte_block(4)
    xto_block(3)
    stageA(0)
    stageA(1)
    conv1(3)
    stageB(0)
    stageA(2)
    conv1(4)
    stageB(1)
    stageA(3)
    stageB(2)

    # ---- main pipelined loop ----
    for it in range(NT):
        if it < 3:
            conv1(it + 5)
        if it % 4 == 2 and it // 4 + 2 <= 7:
            conv2(it // 4 + 2)
        if it + 4 < NT:
            stageA(it + 4)
        if it + 3 < NT:
            stageB(it + 3)
        stageC(it)
        if it + 5 <= 33:
            xte_block(it + 5)
        if it + 4 <= 32:
            xto_block(it + 4)
        if it == 28:
            xto_half(33)
    es.pop_all().close()


def _host_prep(inputs):
    def fold(w, g, b, m, v):
        s = g / np.sqrt(v + EPS)
        return (w * s[:, None, None, None]).astype(np.float32), (b - m * s).astype(np.float32)

    comp_w_eff, comp_b_eff = fold(inputs["comp_w"], inputs["comp_g"], inputs["comp_b"],
                                  inputs["comp_m"], inputs["comp_v"])
    enc_w_eff, enc_b_eff = fold(inputs["enc_w"], inputs["enc_g"], inputs["enc_b"],
                                inputs["enc_m"], inputs["enc_v"])
    cw = np.ascontiguousarray(comp_w_eff[:, :, 0, 0].T)          # [192, 64]
    cb = comp_b_eff.reshape(Cm, 1)
    # packed encoder weights [128, 6*E]: cols dx<3 hold dy=0 (rows 0:64)
    # stacked with dy=1 (rows 64:128); cols 3+dx hold dy=2 in rows 0:64.
    ew = np.zeros((128, 6 * E), np.float32)
    for dx in range(3):
        ew[0:64, dx * E:(dx + 1) * E] = enc_w_eff[:, :, 0, dx].T
        ew[64:128, dx * E:(dx + 1) * E] = enc_w_eff[:, :, 1, dx].T
        ew[0:64, (3 + dx) * E:(4 + dx) * E] = enc_w_eff[:, :, 2, dx].T
    eb = enc_b_eff.reshape(E, 1)
    ident = np.eye(128, dtype=np.float32)
    import ml_dtypes
    identb = np.eye(128, dtype=ml_dtypes.bfloat16)
    idx = np.full((128, 104), -1, np.int16)
    for q in range(128):
        wq = q % 64
        for cl in range(4):
            di, dj = cl // 2, cl % 2
            for i in range(K):
                for j in range(K):
                    wt = wq - (j - 2)
                    if 0 <= wt < W:
                        idx[q, cl * 26 + i * 5 + j] = di * 640 + i * 128 + 2 * wt + dj
    return dict(cw=cw, cb=cb, ew=ew, eb=eb, ident=ident, identb=identb, idx=idx)


def kernel(**inputs):
    from concourse.bass_utils import run_bass_kernel_spmd

    inputs = {k: np.asarray(v, dtype=np.float32) if np.asarray(v).dtype != np.int16
              else np.asarray(v) for k, v in inputs.items()}
    w = _host_prep(inputs)
    if "nc" not in _prog_cache:
        _prog_cache["nc"] = _build_program()
    nc = _prog_cache["nc"]
    x = inputs["x"]
    in_maps = [dict(x=np.ascontiguousarray(x[b].reshape(C, H * W)), **w) for b in range(B)]
    res = run_bass_kernel_spmd(nc, in_maps, list(range(B)))
    out = np.stack([res.results[b]["out"].reshape(C, 2 * H, 2 * W) for b in range(B)])
    return out


# revision 14
# speedup vs baseline: 6.4078x; 1.0121x over previous
# CARAFE (content-aware reassembly) Trainium2 Bass kernel, v2.
# Strategy: data-parallel over batch (8 items -> 8 NeuronCores). Per core:
#   - 1x1 compressor conv + folded BN + SiLU entirely via PE fp32r matmuls
#     (N=512 -> 1 cyc/row) + one ACT Silu per tile.
#   - 3x3 encoder conv as 9 accumulating fp32r matmuls + ACT Exp -> e_sb
#     [100ch, pix] channel-major exp(mask) tensor.
#   - Reassembly on PE in bf16: for each 2-row pixel tile and subpixel-class
#     pair, out[c, p] = sum_i xT_block_i^T @ W_i where W_i are [128, 256]
#     banded matrices holding softmax-normalized mask values on diagonals
#     j-2 in {-2..2}. W is built by ONE gpsimd local_scatter per (tile,
#     class-pair) from data produced with zero partition-shifts:
#     PE-transposing column-shifted slices of e_sb yields all shifted mask
#     columns; constant int16 scatter indices encode tap geometry and edge
#     clipping (idx=-1 drops out-of-image taps, matching x zero-padding).
#   - Softmax normalization: DVE reduce over the transposed (shifted) mask
#     + reciprocal, folded into the scatter data via one strided
#     tensor_tensor (custom APs).
#   - Output accumulated in PSUM [c, (di, h, 2w+dj)], evicted (ACT/DVE) and
#     DMA'd in channel-major order.
import sys
import numpy as np

for _p in ("/opt/trn_rl_repo",):
    if _p not in sys.path:
        sys.path.insert(0, _p)

B, C, Cm, E = 8, 192, 64, 100
H = W = 64
K, S = 5, 2
EPS = 1e-3
NT = 32  # 2-row pixel tiles

_prog_cache = {}


def _build_program(num_devices=8):
    import concourse.mybir as mybir
    import concourse.tile as tile
    from concourse import bacc
    from contextlib import ExitStack

    fp32 = mybir.dt.float32
    nc = bacc.Bacc("TRN2", target_bir_lowering=False, num_devices=num_devices)

    x_d = nc.dram_tensor("x", [C, H * W], mybir.dt.bfloat16, kind="ExternalInput").ap()
    cw_d = nc.dram_tensor("cw", [C, Cm], mybir.dt.bfloat16, kind="ExternalInput").ap()
    cb_d = nc.dram_tensor("cb", [Cm, 1], fp32, kind="ExternalInput").ap()
    ew_d = nc.dram_tensor("ew", [128, 6 * E], mybir.dt.bfloat16, kind="ExternalInput").ap()
    eb_d = nc.dram_tensor("eb", [E, 1], fp32, kind="ExternalInput").ap()
    idb_d = nc.dram_tensor("identb", [128, 128], mybir.dt.bfloat16, kind="ExternalInput").ap()
    idx_d = nc.dram_tensor("idx", [128, 104], mybir.dt.int16, kind="ExternalInput").ap()
    out_d = nc.dram_tensor("out", [C, H, 2, 2 * W], fp32, kind="ExternalOutput").ap()

    es = ExitStack()
    with tile.TileContext(nc) as tc:
        with es:
            _body(es, tc, nc, mybir,
                  x_d, cw_d, cb_d, ew_d, eb_d, idb_d, idx_d, out_d)
    nc.compile()
    return nc


def _body(es, tc, nc, mybir, x_d, cw_d, cb_d, ew_d, eb_d, idb_d, idx_d, out_d):
    from concourse.ap import AP
    from concourse import library_config

    fp32 = mybir.dt.float32
    bf16 = mybir.dt.bfloat16
    AL = mybir.AluOpType
    AF = mybir.ActivationFunctionType

    consts = es.enter_context(tc.tile_pool(name="consts", bufs=1))
    big = es.enter_context(tc.tile_pool(name="big", bufs=1))

    cw0 = consts.tile([128, Cm], bf16, tag="cw0")
    cw1 = consts.tile([64, Cm], bf16, tag="cw1")
    cb = consts.tile([Cm, 1], fp32, tag="cb")
    ew = consts.tile([128, 6 * E], bf16, tag="ew")
    eb = consts.tile([E, 1], fp32, tag="eb")
    identb = consts.tile([128, 128], bf16, tag="identb")
    idxt = consts.tile([128, 104], mybir.dt.int16, tag="idxt")
    zeroT = consts.tile([128, 192], bf16, tag="zeroT")

    x0 = big.tile([128, H * W], bf16, tag="x0")
    x1 = big.tile([64, H * W], bf16, tag="x1")
    t_pad2 = big.tile([128, 66 * 66], bf16, tag="tpad2")
    e_sb = big.tile([E, H * W + 4], bf16, tag="esb")
    xTe = big.tile([128, 36, C], bf16, tag="xTe")
    xTo = big.tile([128, 36, C], bf16, tag="xTo")

    # PSUM pools (8 banks total: 2+1+2+2+1)
    t5ps = es.enter_context(tc.tile_pool(name="t5ps", bufs=2, space="PSUM"))
    convps = es.enter_context(tc.tile_pool(name="convps", bufs=1, space="PSUM"))
    out0ps = es.enter_context(tc.tile_pool(name="out0ps", bufs=2, space="PSUM"))
    out1ps = es.enter_context(tc.tile_pool(name="out1ps", bufs=2, space="PSUM"))
    xtps = es.enter_context(tc.tile_pool(name="xtps", bufs=1, space="PSUM"))

    matsp = es.enter_context(tc.tile_pool(name="matsp", bufs=3))
    rsump = es.enter_context(tc.tile_pool(name="rsump", bufs=3))
    rinvp = es.enter_context(tc.tile_pool(name="rinvp", bufs=3))
    datap = es.enter_context(tc.tile_pool(name="datap", bufs=3))
    wp = es.enter_context(tc.tile_pool(name="wp", bufs=4))
    stg0p = es.enter_context(tc.tile_pool(name="stg0p", bufs=2))
    stg1p = es.enter_context(tc.tile_pool(name="stg1p", bufs=2))

    nc.gpsimd.load_library(library_config.local_scatter)

    # ---- input DMAs (first x chunk + conv1 weights first) ----
    nc.sync.dma_start(out=x0[:, 0:1024], in_=x_d[0:128, 0:1024])
    nc.sync.dma_start(out=x1[:, 0:1024], in_=x_d[128:192, 0:1024])
    nc.sync.dma_start(out=cw0[:], in_=cw_d[0:128, :])
    nc.sync.dma_start(out=cw1[:], in_=cw_d[128:192, :])
    nc.sync.dma_start(out=cb[:], in_=cb_d)
    for ck in range(1, 4):
        c0 = ck * 1024
        nc.sync.dma_start(out=x0[:, c0:c0 + 1024], in_=x_d[0:128, c0:c0 + 1024])
        nc.sync.dma_start(out=x1[:, c0:c0 + 1024], in_=x_d[128:192, c0:c0 + 1024])
    nc.sync.dma_start(out=ew[:], in_=ew_d)
    nc.sync.dma_start(out=eb[:], in_=eb_d)
    nc.sync.dma_start(out=identb[:], in_=idb_d)
    nc.sync.dma_start(out=idxt[:], in_=idx_d)

    # ---- border memsets ----
    nc.gpsimd.memset(zeroT[:], 0.0)
    tp3 = t_pad2[:].rearrange("c (r z) -> c r z", z=66)
    nc.gpsimd.memset(tp3[0:64, 0, :], 0.0)
    nc.gpsimd.memset(tp3[0:64, 65, :], 0.0)
    nc.gpsimd.memset(tp3[:, :, 0:1], 0.0)
    nc.gpsimd.memset(tp3[:, :, 65:66], 0.0)
    nc.gpsimd.memset(e_sb[:, 0:2], 1.0)
    nc.gpsimd.memset(e_sb[:, H * W + 2:H * W + 4], 1.0)
    # zero x-row border blocks of xT (CARAFE zero padding outside the image)
    for t, b in ((xTe, 1), (xTe, 34), (xTo, 1), (xTo, 33)):
        nc.vector.tensor_copy(t[:, b, :], zeroT[:])

    # ---- helpers ----
    def conv1(nt):
        n0 = nt * 512
        ps = out1ps.tile([Cm, 512], fp32, name="c1ps", tag="o1")
        nc.tensor.matmul(ps[:], cw0[:], x0[:, n0:n0 + 512],
                         start=True, stop=False)
        nc.tensor.matmul(ps[:], cw1[:], x1[:, n0:n0 + 512],
                         start=False, stop=True)
        psv = ps[:].rearrange("c (r z) -> c r z", z=64)
        # lower half holds t rows r, upper half t rows r+1 (row-pair packing
        # for the dy in {0,1} encoder taps)
        nc.scalar.activation(out=tp3[0:64, nt * 8 + 1: nt * 8 + 9, 1:65],
                             in_=psv, func=AF.Silu, bias=cb[:], scale=1.0)
        nc.scalar.activation(out=tp3[64:128, nt * 8: nt * 8 + 8, 1:65],
                             in_=psv, func=AF.Silu, bias=cb[:], scale=1.0)

    def conv2(nt):
        r0 = nt * 8
        ps = convps.tile([E, 512], fp32, name="c2ps", tag="conv")
        for dx in range(3):
            rhs = tp3[:, r0: r0 + 8, dx: dx + 64]
            nc.tensor.matmul(ps[:], ew[:, dx * E:(dx + 1) * E], rhs,
                             start=(dx == 0), stop=False)
        for dx in range(3):
            rhs = tp3[0:64, r0 + 2: r0 + 10, dx: dx + 64]
            nc.tensor.matmul(ps[:], ew[:, (3 + dx) * E:(4 + dx) * E][0:64, :],
                             rhs, start=False, stop=(dx == 2))
        nc.scalar.activation(out=e_sb[:, 2 + r0 * 64: 2 + r0 * 64 + 512], in_=ps[:],
                             func=AF.Exp, bias=eb[:], scale=1.0)

    nxt = [0]  # alternate eviction engine for xT blocks

    def _xt_evict(dst, src):
        if nxt[0] % 2 == 0:
            nc.scalar.copy(out=dst, in_=src)
        else:
            nc.vector.tensor_copy(dst, src)
        nxt[0] += 1

    def xte_block(be):
        px0 = 128 * (be - 2)
        pt = xtps.tile([128, C], bf16, name="xtpt", tag="xt")
        nc.tensor.matmul(pt[:, 0:128], x0[:, px0:px0 + 128], identb[:],
                         is_transpose=True, skip_group_check=True)
        nc.tensor.matmul(pt[:, 128:192], x1[:, px0:px0 + 128],
                         identb[0:64, 0:64], is_transpose=True,
                         skip_group_check=True)
        _xt_evict(xTe[:, be, :], pt[:])

    def xto_block(bo):
        px0 = 128 * (bo - 2) + 64
        pt = xtps.tile([128, C], bf16, name="xopt", tag="xt")
        nc.tensor.matmul(pt[:, 0:128], x0[:, px0:px0 + 128], identb[:],
                         is_transpose=True, skip_group_check=True)
        nc.tensor.matmul(pt[:, 128:192], x1[:, px0:px0 + 128],
                         identb[0:64, 0:64], is_transpose=True,
                         skip_group_check=True)
        _xt_evict(xTo[:, bo, :], pt[:])

    def xto_half(bo):
        pt = xtps.tile([128, C], bf16, name="xhpt", tag="xt")
        if bo == 1:  # rows (-1, 0): only upper 64 partitions hold row 0
            cols, prt = slice(0, 64), slice(64, 128)
        else:  # bo == 33: rows (63, 64): lower 64 partitions hold row 63
            cols, prt = slice(4032, 4096), slice(0, 64)
        nc.tensor.matmul(pt[prt, 0:128], x0[:, cols], identb[:],
                         is_transpose=True, skip_group_check=True)
        nc.tensor.matmul(pt[prt, 128:192], x1[:, cols],
                         identb[0:64, 0:64], is_transpose=True,
                         skip_group_check=True)
        _xt_evict(xTo[prt, bo, :], pt[prt, :])

    data_tiles = {}
    w_tiles = {}

    def stageA(ti):
        # 5 shifted transposes of e_sb -> T5 [128, 5, 100] (psum), then
        # normalize into bf16 scatter data [128, 4cl, 26].
        p0 = 128 * ti
        t5 = t5ps.tile([128, 500], bf16, name="t5", tag="t5")
        for d in range(5):
            s = p0 + 4 - d
            nc.tensor.matmul(t5[:, d * 100:(d + 1) * 100],
                             e_sb[:, s:s + 128], identb[0:E, 0:E],
                             is_transpose=True, skip_group_check=True)
        mats = matsp.tile([128, 500], bf16, name="mats", tag="mats")
        nc.scalar.copy(out=mats[:], in_=t5[:])
        rsum = rsump.tile([128, 20], fp32, name="rsum", tag="rsum")
        red_in = AP(t5[:].tensor, 0, [[500, 128], [100, 5], [1, 4], [4, 25]])
        nc.vector.tensor_reduce(out=rsum[:].rearrange("q (d c) -> q d c", c=4),
                                in_=red_in, axis=mybir.AxisListType.X, op=AL.add)
        rinv = rinvp.tile([128, 20], fp32, name="rinv", tag="rinv")
        nc.vector.reciprocal(rinv[:], rsum[:])
        data = datap.tile([128, 4, 26], bf16, name="data", tag="data")
        # data[q, cl, i*5+j] = mats[q, 104j + 20i + cl] * rinv[q, 4j + cl]
        in0 = AP(mats[:].tensor, 0, [[500, 128], [1, 4], [20, 5], [104, 5]])
        in1 = AP(rinv[:].tensor, 0, [[20, 128], [1, 4], [0, 5], [4, 5]])
        outv = AP(data[:].tensor, 0, [[104, 128], [26, 4], [5, 5], [1, 5]])
        nc.vector.tensor_tensor(outv, in0, in1, AL.mult)
        data_tiles[ti] = data

    def stageB(ti):
        # One scatter builds the whole tile's W [128, (di, i, 2w+dj)]: the
        # output-row dimension is implicit in the partition halves (q<64 ->
        # out row h0, q>=64 -> h0+1), so no structurally-zero quadrants.
        data = data_tiles.pop(ti)
        w = wp.tile([128, 2, 5, 128], bf16, name="w", tag="w")
        nc.gpsimd.local_scatter(
            out_ap=w[:].rearrange("q a b c -> q (a b c)"),
            data_ap=data[:].rearrange("q a b -> q (a b)"),
            idxs_ap=idxt[:],
            channels=128, num_elems=1280, num_idxs=104)
        w_tiles[ti] = w

    def stageC(ti):
        w = w_tiles[ti]
        for ck, (c0, cp, psp, stgp, ev) in enumerate(
                ((0, 128, out0ps, stg0p, "act"), (128, 64, out1ps, stg1p, "dve"))):
            # psum columns in output row-major order (h, di, w'): K=64
            # matmuls pair partition half hq of the x block with the same
            # half of W (out row h0+hq).
            ps = psp.tile([cp, 512], fp32, name=f"ops{ck}", tag=f"o{ck}")
            for hq, qs in ((0, slice(0, 64)), (1, slice(64, 128))):
                for di in range(2):
                    n0 = hq * 256 + di * 128
                    for i in range(5):
                        if i % 2 == 0:
                            blk = xTe[qs, ti + 1 + i // 2, c0:c0 + cp]
                        else:
                            blk = xTo[qs, ti + 1 + (i - 1) // 2, c0:c0 + cp]
                        nc.tensor.matmul(ps[:, n0:n0 + 128], blk, w[qs, di, i, :],
                                         start=(i == 0), stop=(i == 4),
                                         skip_group_check=True)
            stg = stgp.tile([cp, 512], fp32, name=f"stg{ck}", tag=f"s{ck}")
            if ev == "act":
                nc.scalar.copy(out=stg[:], in_=ps[:])
            else:
                nc.vector.tensor_copy(stg[:], ps[:])
            nc.sync.dma_start(
                out=out_d[c0:c0 + cp, 2 * ti:2 * ti + 2, :, :].rearrange(
                    "c h a w -> c (h a w)"),
                in_=stg[:])
        del w_tiles[ti]

    # ---- prologue ----
    # start the reassembly pipeline as soon as conv coverage allows; conv1
    # runs ahead of conv2 by one tile (all Silu first would idle the PE on
    # input DMAs).
    conv1(0)
    conv1(1)
    xte_block(2)
    xte_block(3)
    xto_half(1)
    xto_block(2)
    conv2(0)
    conv1(2)
    conv2(1)
    xte_block(4)
    xto_block(3)
    stageA(0)
    stageA(1)
    conv1(3)
    stageB(0)
    stageA(2)
    conv1(4)
    stageB(1)
    stageA(3)
    stageB(2)

    # ---- main pipelined loop ----
    for it in range(NT):
        if it < 3:
            conv1(it + 5)
        if it % 4 == 2 and it // 4 + 2 <= 7:
            conv2(it // 4 + 2)
        if it + 4 < NT:
            stageA(it + 4)
        if it + 3 < NT:
            stageB(it + 3)
        stageC(it)
        if it + 5 <= 33:
            xte_block(it + 5)
        if it + 4 <= 32:
            xto_block(it + 4)
        if it == 28:
            xto_half(33)
    es.pop_all().close()


def _host_prep(inputs):
    def fold(w, g, b, m, v):
        s = g / np.sqrt(v + EPS)
        return (w * s[:, None, None, None]).astype(np.float32), (b - m * s).astype(np.float32)

    comp_w_eff, comp_b_eff = fold(inputs["comp_w"], inputs["comp_g"], inputs["comp_b"],
                                  inputs["comp_m"], inputs["comp_v"])
    enc_w_eff, enc_b_eff = fold(inputs["enc_w"], inputs["enc_g"], inputs["enc_b"],
                                inputs["enc_m"], inputs["enc_v"])
    cw = np.ascontiguousarray(comp_w_eff[:, :, 0, 0].T)          # [192, 64]
    cb = comp_b_eff.reshape(Cm, 1)
    # packed encoder weights [128, 6*E]: cols dx<3 hold dy=0 (rows 0:64)
    # stacked with dy=1 (rows 64:128); cols 3+dx hold dy=2 in rows 0:64.
    ew = np.zeros((128, 6 * E), np.float32)
    for dx in range(3):
        ew[0:64, dx * E:(dx + 1) * E] = enc_w_eff[:, :, 0, dx].T
        ew[64:128, dx * E:(dx + 1) * E] = enc_w_eff[:, :, 1, dx].T
        ew[0:64, (3 + dx) * E:(4 + dx) * E] = enc_w_eff[:, :, 2, dx].T
    eb = enc_b_eff.reshape(E, 1)
    import ml_dtypes
    bf = ml_dtypes.bfloat16
    cw = cw.astype(bf)
    ew = ew.astype(bf)
    identb = np.eye(128, dtype=bf)
    idx = np.full((128, 104), -1, np.int16)
    for q in range(128):
        wq = q % 64
        for cl in range(4):
            di, dj = cl // 2, cl % 2
            for i in range(K):
                for j in range(K):
                    wt = wq - (j - 2)
                    if 0 <= wt < W:
                        idx[q, cl * 26 + i * 5 + j] = di * 640 + i * 128 + 2 * wt + dj
    return dict(cw=cw, cb=cb, ew=ew, eb=eb, identb=identb, idx=idx)


def kernel(**inputs):
    from concourse.bass_utils import run_bass_kernel_spmd

    inputs = {k: np.asarray(v, dtype=np.float32) if np.asarray(v).dtype != np.int16
              else np.asarray(v) for k, v in inputs.items()}
    w = _host_prep(inputs)
    if "nc" not in _prog_cache:
        _prog_cache["nc"] = _build_program()
    nc = _prog_cache["nc"]
    x = inputs["x"]
    import ml_dtypes
    xb = np.ascontiguousarray(x.reshape(B, C, H * W)).astype(ml_dtypes.bfloat16)
    in_maps = [dict(x=xb[b], **w) for b in range(B)]
    res = run_bass_kernel_spmd(nc, in_maps, list(range(B)))
    out = np.stack([res.results[b]["out"].reshape(C, 2 * H, 2 * W) for b in range(B)])
    return out
